# revision 32
# baseline (speedup 1.0000x reference)
"""AMMLinear (VQ codebook) forward on 8 TRN2 NeuronCores.

The straight-through estimator makes the forward VALUE exactly
    out[n, o] = sum_c lut_dq[c, argmin_k dist(x_cn, cent_ck), o] + bias[o]
with lut = centroids @ weight (per codebook) and lut_dq a global-scale int8
quantize-dequantize of lut.  The softmax/attention path only shapes gradients.

Sharding: tokens (BN=4096 -> 512/core) for the score/argmin phase, output
features (4096 -> 512/core) for the lut/gather phase.  One AllGather moves the
bf16 argmin indices (64KB/core), one AllReduce-max the quantization scale.
Every core then expands all 4096 tokens' one-hot codes locally (replication
DMA + is_equal) and computes its o-shard of the gather matmul.
Output is assembled host-side by concatenating the per-core o-shards.

Numerics: scores and lut are computed as 3-pass bf16 hi/lo matmuls
(a*b ~ ah*bh + ah*bl + al*bh, fp32 PSUM accumulate, ~2^-18/product), the
gather matmul in bf16 (exact one-hots, bf16-rounded lut_dq), output in bf16.
Measured end-to-end rel err ~5e-3 against the fp32 reference.
"""

import numpy as np

import concourse.bass as bass
import concourse.mybir as mybir
import concourse.tile as tile
import concourse.bass_isa as bass_isa
from concourse import bacc
from concourse.bass_utils import run_bass_kernel_spmd
from concourse.masks import make_identity

F32 = mybir.dt.float32
BF16 = mybir.dt.bfloat16

N_CORES = 8
NC, K, IN_F, OUT_F = 64, 16, 4096, 4096
SUBV = IN_F // NC          # 64
BN = 4096                  # 2*2048 tokens
TOK = BN // N_CORES        # 512 tokens per core
NT = TOK // 128            # 4 token tiles per core
NPAIR = NC // 2            # 32 codebook pairs
CK = NC * K                # 1024 (codebook,centroid) flat index
NCHUNK = CK // 128         # 8 contraction chunks
OSH = OUT_F // N_CORES     # 512 out features per core
MAGIC = 12582912.0         # 1.5 * 2**23: fp32 round-to-nearest-even trick
BIG = 1024.0

_CACHE = {}


def _build():
    nc = bacc.Bacc("TRN2", target_bir_lowering=False, debug=False,
                   num_devices=N_CORES)

    # xhl[:, 0:TOK] = bf16 hi of x^T shard, [:, TOK:2*TOK] = bf16 lo
    xhl = nc.declare_dram_parameter("xhl", [IN_F, 2 * TOK], BF16,
                                    isOutput=False)
    # whl[:, 0:OSH] = bf16 hi of weight o-shard, [:, OSH:2*OSH] = lo
    whl = nc.declare_dram_parameter("whl", [IN_F, 2 * OSH], BF16,
                                    isOutput=False)
    cbd_h = nc.declare_dram_parameter("cbd_h", [128, CK], BF16, isOutput=False)
    cbd_l = nc.declare_dram_parameter("cbd_l", [128, CK], BF16, isOutput=False)
    c2r = nc.declare_dram_parameter("c2r", [128, CK], F32, isOutput=False)
    iotar = nc.declare_dram_parameter("iotar", [128, 128], F32, isOutput=False)
    biasr = nc.declare_dram_parameter("biasr", [128, OSH], F32, isOutput=False)
    kcol = nc.declare_dram_parameter("kcol", [128, 1], F32, isOutput=False)
    out = nc.declare_dram_parameter("out", [BN, OSH], BF16, isOutput=True)

    with tile.TileContext(nc) as tc:
        with (
            tc.tile_pool(name="consts", bufs=1) as constp,
            tc.tile_pool(name="xt", bufs=6) as xtp,
            tc.tile_pool(name="xpre", bufs=12) as xprep,
            tc.tile_pool(name="wt", bufs=6) as wp,
            tc.tile_pool(name="xct", bufs=3) as xctp,
            tc.tile_pool(name="work", bufs=2) as workp,
            tc.tile_pool(name="stg", bufs=8) as stgp,
            tc.tile_pool(name="strep", bufs=4) as strepp,
            tc.tile_pool(name="half", bufs=32) as halfp,
            tc.tile_pool(name="lut", bufs=1) as lutp,
            tc.tile_pool(name="outs", bufs=2) as outp,
            tc.tile_pool(name="ps", bufs=8, space="PSUM") as psp,
            tc.tile_pool(name="dram", bufs=1, space="DRAM") as dramp,
        ):
            # ---- constants -------------------------------------------------
            cbdh_sb = constp.tile([128, CK], BF16, tag="cbdh_sb")
            nc.sync.dma_start(out=cbdh_sb[:, :], in_=cbd_h.ap()[:, :])
            cbdl_sb = constp.tile([128, CK], BF16, tag="cbdl_sb")
            nc.sync.dma_start(out=cbdl_sb[:, :], in_=cbd_l.ap()[:, :])
            c2_sb = constp.tile([128, CK], F32, tag="c2_sb")
            nc.sync.dma_start(out=c2_sb[:, :], in_=c2r.ap()[:, :])
            iota_sb = constp.tile([128, 128], F32, tag="iota_sb")
            nc.sync.dma_start(out=iota_sb[:, :], in_=iotar.ap()[:, :])
            identb = constp.tile([128, 128], BF16, tag="identb")
            make_identity(nc, identb[:, :])
            identf = constp.tile([128, 128], F32, tag="identf")
            make_identity(nc, identf[:, :])

            # DRAM scratch for collectives
            kt_bounce1 = dramp.tile([NC // 2, TOK], BF16, tag="kt_bounce1")
            kt_bounce2 = dramp.tile([NC // 2, TOK], BF16, tag="kt_bounce2")
            kt_all1 = dramp.tile([N_CORES * NC // 2, TOK], BF16, tag="kt_all1")
            kt_all2 = dramp.tile([N_CORES * NC // 2, TOK], BF16, tag="kt_all2")
            mx_in = dramp.tile([1, 16], F32, tag="mx_in")
            mx_out = dramp.tile([1, 16], F32, tag="mx_out")

            # ---- phase A: lut = centroids @ weight (3-pass bf16 hi/lo) ----
            # Runs first so the absmax -> AllReduce(max) chain is issued as
            # early as possible (the collective queue is FIFO: the first
            # collective blocks the second until it completes).
            lut_sb = lutp.tile([128, NCHUNK * OSH], F32, tag="lut")
            mx8 = constp.tile([128, NCHUNK], F32, tag="mx8")
            for j in range(NCHUNK):
                ps_lut = psp.tile([128, OSH], F32, tag="ps")
                for mcol in range(4):
                    p = 4 * j + mcol
                    w_t = wp.tile([128, 2 * OSH], BF16, tag="wt")
                    nc.scalar.dma_start(
                        out=w_t[:, :],
                        in_=whl.ap()[128 * p:128 * (p + 1), :])
                    passes = [(cbdh_sb, 0), (cbdh_sb, OSH), (cbdl_sb, 0)]
                    for i, (cb, off) in enumerate(passes):
                        nc.tensor.matmul(
                            ps_lut[32 * mcol:32 * (mcol + 1), :],
                            lhsT=cb[:, 32 * p:32 * (p + 1)],
                            rhs=w_t[:, off:off + OSH],
                            start=(i == 0), stop=(i == 2),
                            tile_position=(0, 32 * mcol),
                        )
                nc.vector.tensor_copy(
                    out=lut_sb[:, OSH * j:OSH * (j + 1)], in_=ps_lut[:, :])
                nc.vector.tensor_reduce(
                    out=mx8[:, j:j + 1],
                    in_=lut_sb[:, OSH * j:OSH * (j + 1)],
                    axis=mybir.AxisListType.X, op=mybir.AluOpType.max,
                    apply_absolute_value=True,
                )

            # ---- early x prefetch (gpsimd queue, parallel to whl/sync) -----
            xpre_tiles = []
            for p in range(12):
                xp_t = xprep.tile([128, 2 * TOK], BF16, tag="xpre",
                                  name=f"xpre{p}")
                nc.gpsimd.dma_start(out=xp_t[:, :],
                                    in_=xhl.ap()[128 * p:128 * (p + 1), :])
                xpre_tiles.append(xp_t)

            # ---- phase B: global absmax -> AllReduce(max) -> scale ---------
            mxl = constp.tile([128, 1], F32, tag="mxl")
            nc.vector.tensor_reduce(
                out=mxl[:, :], in_=mx8[:, :], axis=mybir.AxisListType.X,
                op=mybir.AluOpType.max,
            )
            mxp = constp.tile([128, 1], F32, tag="mxp")
            nc.gpsimd.partition_all_reduce(
                mxp[:, :], mxl[:, :], channels=128,
                reduce_op=bass_isa.ReduceOp.max,
            )
            zrow = constp.tile([1, 16], F32, tag="zrow")
            nc.vector.memset(zrow[:, :], 0.0)
            nc.vector.tensor_copy(out=zrow[0:1, 0:1], in_=mxp[0:1, 0:1])
            nc.gpsimd.dma_start(out=mx_in[:, :], in_=zrow[:, :])
            nc.gpsimd.collective_compute(
                "AllReduce",
                mybir.AluOpType.max,
                replica_groups=[list(range(N_CORES))],
                ins=[mx_in.opt()],
                outs=[mx_out.opt()],
            )
            gmax_row = constp.tile([1, 16], F32, tag="gmax_row")
            nc.gpsimd.dma_start(out=gmax_row[:, :], in_=mx_out[:, :])
            gmax = constp.tile([128, 1], F32, tag="gmax")
            nc.gpsimd.partition_broadcast(gmax[:, :], gmax_row[0:1, 0:1])
            # s = gmax/127 and 1/s via reciprocal (DVE has no divide); the
            # <=2ulp drift is far inside the error budget.
            s_col = constp.tile([128, 1], F32, tag="s_col")
            nc.vector.tensor_scalar(
                out=s_col[:, :], in0=gmax[:, :],
                scalar1=float(np.float32(1.0) / np.float32(127.0)),
                scalar2=None, op0=mybir.AluOpType.mult,
            )
            rgmax = constp.tile([128, 1], F32, tag="rgmax")
            nc.vector.reciprocal(rgmax[:, :], gmax[:, :])
            inv_s = constp.tile([128, 1], F32, tag="inv_s")
            nc.vector.tensor_scalar(
                out=inv_s[:, :], in0=rgmax[:, :], scalar1=127.0, scalar2=None,
                op0=mybir.AluOpType.mult,
            )

            # ---- phase C: scores + incremental argmin ----------------------
            # The DVE executes its stream in order, so the argmin chain for
            # chunk j is emitted AFTER chunk j+1's PSUM evict: the evict is
            # never stuck behind argmin work and PE transposes stay fed.
            kminbig = workp.tile([128, NT * NC], F32, tag="kminbig", bufs=1)
            ps_trs = [None] * NCHUNK

            def argmin_chunk(j):
                ps_tr4 = ps_trs[j]
                ssl = workp.tile([128, TOK], F32, tag="ssl", name=f"ssl{j}")
                nc.vector.scalar_tensor_tensor(
                    out=ssl[:, :].rearrange("p (t f) -> p t f", f=128),
                    in0=ps_tr4[:, :].rearrange("p (t f) -> p t f", f=128),
                    in1=c2_sb[:, 128 * j:128 * (j + 1)].unsqueeze(1)
                        .broadcast_to([128, NT, 128]),
                    scalar=-2.0,
                    op0=mybir.AluOpType.mult,
                    op1=mybir.AluOpType.add,
                )
                ssl4 = ssl[:, :].rearrange("p (t c k) -> p t c k", k=K, c=8)
                m32 = workp.tile([128, NT * 8], F32, tag="m32", name=f"m32_{j}")
                nc.vector.tensor_reduce(
                    out=m32[:, :], in_=ssl4, axis=mybir.AxisListType.X,
                    op=mybir.AluOpType.min,
                )
                eq = workp.tile([128, TOK], F32, tag="eq", name=f"eq{j}")
                nc.vector.tensor_tensor(
                    out=eq[:, :].rearrange("p (t c k) -> p t c k", k=K, c=8),
                    in0=ssl4,
                    in1=m32[:, :].rearrange("p (t c) -> p t c", c=8)
                        .unsqueeze(3).broadcast_to([128, NT, 8, K]),
                    op=mybir.AluOpType.is_equal,
                )
                cand = workp.tile([128, TOK], F32, tag="cand", name=f"cand{j}")
                nc.vector.scalar_tensor_tensor(
                    out=cand[:, :].rearrange("p (t f) -> p t f", f=128),
                    in0=eq[:, :].rearrange("p (t f) -> p t f", f=128),
                    in1=iota_sb[:, :].unsqueeze(1)
                        .broadcast_to([128, NT, 128]),
                    scalar=-BIG,
                    op0=mybir.AluOpType.mult, op1=mybir.AluOpType.add,
                )
                nc.vector.tensor_reduce(
                    out=kminbig[:, :].rearrange(
                        "p (t c) -> p t c", c=NC)[:, :, 8 * j:8 * (j + 1)],
                    in_=cand[:, :].rearrange("p (t c k) -> p t c k", k=K, c=8),
                    axis=mybir.AxisListType.X, op=mybir.AluOpType.min,
                )

            def ship_half(half):
                # kmin [n, c-half] -> bf16 -> transpose -> AllGather.
                # Collective order (AR, AG1, AG2) matches input readiness so
                # the FIFO collective queue never head-of-line blocks.
                c0 = (NC // 2) * half
                kth = constp.tile([NC // 2, TOK], BF16, tag=f"kt_sb{half}",
                                  name=f"kt_sb{half}")
                for t in range(NT):
                    kminb = workp.tile([128, NC // 2], BF16, tag="kminb",
                                       name=f"kminb{half}_{t}")
                    nc.vector.tensor_scalar_add(
                        kminb[:, :].rearrange("p (c j) -> p c j", j=4),
                        kminbig[:, NC * t + c0:NC * t + c0 + NC // 2]
                            .rearrange("p (j c) -> p j c", c=8)
                            .transpose([0, 2, 1]),
                        BIG)
                    ps_kt = psp.tile([NC // 2, 128], BF16, tag="ps",
                                     name=f"ps_kt{half}_{t}")
                    nc.tensor.transpose(ps_kt[:, :], kminb[:, :],
                                        identb[:, :])
                    nc.vector.tensor_copy(
                        out=kth[:, 128 * t:128 * (t + 1)], in_=ps_kt[:, :])
                bnc = kt_bounce1 if half == 0 else kt_bounce2
                gat = kt_all1 if half == 0 else kt_all2
                nc.gpsimd.dma_start(out=bnc[:, :], in_=kth[:, :])
                nc.gpsimd.collective_compute(
                    "AllGather",
                    mybir.AluOpType.bypass,
                    replica_groups=[list(range(N_CORES))],
                    ins=[bnc.opt()],
                    outs=[gat.opt()],
                )

            for j in range(NCHUNK):
                ps_xct = psp.tile([128, TOK], F32, tag="ps", name=f"ps_xct{j}")
                for mcol in range(4):
                    p = 4 * j + mcol
                    if p < 12:
                        xt_t = xpre_tiles[p]
                    else:
                        xt_t = xtp.tile([128, 2 * TOK], BF16, tag="xt")
                        nc.sync.dma_start(
                            out=xt_t[:, :],
                            in_=xhl.ap()[128 * p:128 * (p + 1), :])
                    passes = [(cbdh_sb, 0), (cbdh_sb, TOK), (cbdl_sb, 0)]
                    for i, (cb, off) in enumerate(passes):
                        nc.tensor.matmul(
                            ps_xct[32 * mcol:32 * (mcol + 1), :],
                            lhsT=cb[:, 32 * p:32 * (p + 1)],
                            rhs=xt_t[:, off:off + TOK],
                            start=(i == 0), stop=(i == 2),
                            tile_position=(0, 32 * mcol),
                        )
                xct_sb = xctp.tile([128, TOK], F32, tag="xct")
                nc.vector.tensor_copy(out=xct_sb[:, :], in_=ps_xct[:, :])
                ps_tr4 = psp.tile([128, TOK], F32, tag="ps", name=f"ps_tr4_{j}")
                for t in range(NT):
                    nc.tensor.transpose(
                        ps_tr4[:, 128 * t:128 * (t + 1)],
                        xct_sb[:, 128 * t:128 * (t + 1)],
                        identf[:, :],
                    )
                ps_trs[j] = ps_tr4
                if j > 0:
                    argmin_chunk(j - 1)
                    ps_trs[j - 1] = None
                if j == 4:
                    ship_half(0)
            argmin_chunk(NCHUNK - 1)
            ship_half(1)

            # ---- phase 5: quantize-dequantize lut -> bf16 ------------------
            lutdq = lutp.tile([128, NCHUNK * OSH], BF16, tag="lutdq")
            for j in range(NCHUNK):
                qm = wp.tile([128, OSH], F32, tag="qm")
                nc.vector.tensor_scalar(
                    out=qm[:, :], in0=lut_sb[:, OSH * j:OSH * (j + 1)],
                    scalar1=inv_s[:, 0:1], scalar2=MAGIC,
                    op0=mybir.AluOpType.mult, op1=mybir.AluOpType.add,
                )
                nc.vector.tensor_scalar(
                    out=lutdq[:, OSH * j:OSH * (j + 1)], in0=qm[:, :],
                    scalar1=MAGIC, scalar2=s_col[:, 0:1],
                    op0=mybir.AluOpType.subtract, op1=mybir.AluOpType.mult,
                )

            bias_sb = constp.tile([128, OSH], F32, tag="bias_sb")
            nc.sync.dma_start(out=bias_sb[:, :], in_=biasr.ap()[:, :])
            kcol_sb = constp.tile([128, 1], F32, tag="kcol_sb")
            nc.sync.dma_start(out=kcol_sb[:, :], in_=kcol.ap()[:, :])

            # ---- phase 6: expand one-hots + gather matmul ------------------
            # Two half-contractions: chunks 0-3 right after AG1 (overlapping
            # AG2), chunks 4-7 after AG2; bias folded into the first half.
            # One 4D-AP replication DMA + one is_equal per (half, rank).
            half_tiles = {}
            for half in range(2):
                gat = kt_all1 if half == 0 else kt_all2
                for r in range(N_CORES):
                    rep = strepp.tile([128, 4 * TOK], BF16, tag="strep")
                    nc.scalar.dma_start(
                        out=rep[:, :],
                        in_=gat[32 * r:32 * (r + 1), :]
                            .rearrange("(c j) n -> c (j n)", j=4)
                            .unsqueeze(1).broadcast_to([8, K, 4 * TOK]),
                    )
                    stg_t = stgp.tile([128, 4 * TOK], BF16, tag="stg")
                    nc.vector.tensor_scalar(
                        out=stg_t[:, :], in0=rep[:, :],
                        scalar1=kcol_sb[:, 0:1], scalar2=None,
                        op0=mybir.AluOpType.is_equal,
                    )
                    o_sb = None
                    if half == 1:
                        o_sb = outp.tile([128, NT * OSH], BF16, tag="outsb")
                    for t in range(NT):
                        ps_o = psp.tile([128, OSH], F32, tag="ps")
                        for jj in range(NCHUNK // 2):
                            jglob = 4 * half + jj
                            nc.tensor.matmul(
                                ps_o[:, :],
                                lhsT=stg_t[:, TOK * jj + 128 * t:
                                           TOK * jj + 128 * (t + 1)],
                                rhs=lutdq[:, OSH * jglob:OSH * (jglob + 1)],
                                start=(jj == 0), stop=(jj == NCHUNK // 2 - 1),
                            )
                        if half == 0:
                            hs = halfp.tile([128, OSH], BF16, tag="hsum",
                                            name=f"hs{r}_{t}")
                            nc.vector.tensor_tensor(
                                out=hs[:, :], in0=ps_o[:, :], in1=bias_sb[:, :],
                                op=mybir.AluOpType.add,
                            )
                            half_tiles[(r, t)] = hs
                        else:
                            nc.vector.tensor_tensor(
                                out=o_sb[:, OSH * t:OSH * (t + 1)],
                                in0=ps_o[:, :], in1=half_tiles[(r, t)][:, :],
                                op=mybir.AluOpType.add,
                            )
                    if half == 1:
                        nc.sync.dma_start(
                            out=out.ap()[TOK * r:TOK * (r + 1), :]
                                .rearrange("(t p) o -> p t o", p=128),
                            in_=o_sb[:, :].rearrange("p (t o) -> p t o", o=OSH),
                        )

    nc.compile()
    return nc


def _prep_inputs(x, centroids, weight, bias):
    import ml_dtypes

    x = np.ascontiguousarray(np.asarray(x, dtype=np.float32)).reshape(BN, IN_F)
    cent = np.asarray(centroids, dtype=np.float32)
    w = np.asarray(weight, dtype=np.float32)
    bias = np.asarray(bias, dtype=np.float32)

    c2 = (cent ** 2).sum(axis=-1).reshape(CK)  # [1024] flat (c,k)
    c2r = np.ascontiguousarray(np.broadcast_to(c2, (128, CK)))
    iota = np.tile(np.arange(K, dtype=np.float32), 8)
    iotar = np.ascontiguousarray(np.broadcast_to(iota, (128, 128)))
    cbd = np.zeros((128, CK), np.float32)
    for p in range(NPAIR):
        cbd[0:SUBV, 32 * p:32 * p + K] = cent[2 * p].T
        cbd[SUBV:128, 32 * p + K:32 * p + 2 * K] = cent[2 * p + 1].T
    cbd_h = cbd.astype(ml_dtypes.bfloat16)
    cbd_l = (cbd - cbd_h.astype(np.float32)).astype(ml_dtypes.bfloat16)
    kcol = np.ascontiguousarray(
        (np.arange(128, dtype=np.float32) % K).reshape(128, 1))

    in_maps = []
    for r in range(N_CORES):
        xT_r = np.ascontiguousarray(x[TOK * r:TOK * (r + 1)].T)
        xh_r = xT_r.astype(ml_dtypes.bfloat16)
        xl_r = (xT_r - xh_r.astype(np.float32)).astype(ml_dtypes.bfloat16)
        xhl_r = np.ascontiguousarray(np.concatenate([xh_r, xl_r], axis=1))
        w_r = np.ascontiguousarray(w[:, :, OSH * r:OSH * (r + 1)]).reshape(
            IN_F, OSH)
        w_h = w_r.astype(ml_dtypes.bfloat16)
        w_l = (w_r - w_h.astype(np.float32)).astype(ml_dtypes.bfloat16)
        whl_r = np.ascontiguousarray(np.concatenate([w_h, w_l], axis=1))
        bias_r = np.ascontiguousarray(
            np.broadcast_to(bias[OSH * r:OSH * (r + 1)], (128, OSH)))
        in_maps.append({
            "xhl": xhl_r, "whl": whl_r, "cbd_h": cbd_h, "cbd_l": cbd_l,
            "c2r": c2r, "iotar": iotar, "biasr": bias_r, "kcol": kcol,
        })
    return in_maps


def kernel(x, centroids, weight, inverse_temperature_logit, bias,
           **_unused) -> np.ndarray:
    if "nc" not in _CACHE:
        _CACHE["nc"] = _build()
    nc = _CACHE["nc"]
    in_maps = _prep_inputs(x, centroids, weight, bias)
    res = run_bass_kernel_spmd(nc, in_maps, core_ids=list(range(N_CORES)))
    out = np.concatenate(
        [res.results[r]["out"].astype(np.float32) for r in range(N_CORES)],
        axis=1)
    return out.reshape(2, BN // 2, OUT_F)


# revision 33
# speedup vs baseline: 1.0659x; 1.0659x over previous
"""AMMLinear (VQ codebook) forward on 8 TRN2 NeuronCores.

The straight-through estimator makes the forward VALUE exactly
    out[n, o] = sum_c lut_dq[c, argmin_k dist(x_cn, cent_ck), o] + bias[o]
with lut = centroids @ weight (per codebook) and lut_dq a global-scale int8
quantize-dequantize of lut.  The softmax/attention path only shapes gradients.

Sharding: tokens (BN=4096 -> 512/core) for the score/argmin phase, output
features (4096 -> 512/core) for the lut/gather phase.  One AllGather moves the
bf16 argmin indices (64KB/core), one AllReduce-max the quantization scale.
Every core then expands all 4096 tokens' one-hot codes locally (replication
DMA + is_equal) and computes its o-shard of the gather matmul.
Output is assembled host-side by concatenating the per-core o-shards.

Numerics: scores and lut are computed as 3-pass bf16 hi/lo matmuls
(a*b ~ ah*bh + ah*bl + al*bh, fp32 PSUM accumulate, ~2^-18/product), the
gather matmul in bf16 (exact one-hots, bf16-rounded lut_dq), output in bf16.
Measured end-to-end rel err ~5e-3 against the fp32 reference.
"""

import numpy as np

import concourse.bass as bass
import concourse.mybir as mybir
import concourse.tile as tile
import concourse.bass_isa as bass_isa
from concourse import bacc
from concourse.bass_utils import run_bass_kernel_spmd
from concourse.masks import make_identity

F32 = mybir.dt.float32
BF16 = mybir.dt.bfloat16

N_CORES = 8
NC, K, IN_F, OUT_F = 64, 16, 4096, 4096
SUBV = IN_F // NC          # 64
BN = 4096                  # 2*2048 tokens
TOK = BN // N_CORES        # 512 tokens per core
NT = TOK // 128            # 4 token tiles per core
NPAIR = NC // 2            # 32 codebook pairs
CK = NC * K                # 1024 (codebook,centroid) flat index
NCHUNK = CK // 128         # 8 contraction chunks
OSH = OUT_F // N_CORES     # 512 out features per core
MAGIC = 12582912.0         # 1.5 * 2**23: fp32 round-to-nearest-even trick
BIG = 1024.0

_CACHE = {}


def _build():
    nc = bacc.Bacc("TRN2", target_bir_lowering=False, debug=False,
                   num_devices=N_CORES)

    # xhl[:, 0:TOK] = bf16 hi of x^T shard, [:, TOK:2*TOK] = bf16 lo
    xhl = nc.declare_dram_parameter("xhl", [IN_F, 2 * TOK], BF16,
                                    isOutput=False)
    # whl[:, 0:OSH] = bf16 hi of weight o-shard, [:, OSH:2*OSH] = lo
    whl = nc.declare_dram_parameter("whl", [IN_F, 2 * OSH], BF16,
                                    isOutput=False)
    cbd_h = nc.declare_dram_parameter("cbd_h", [128, CK], BF16, isOutput=False)
    cbd_l = nc.declare_dram_parameter("cbd_l", [128, CK], BF16, isOutput=False)
    c2r = nc.declare_dram_parameter("c2r", [128, CK], F32, isOutput=False)
    iotar = nc.declare_dram_parameter("iotar", [128, 128], F32, isOutput=False)
    biasr = nc.declare_dram_parameter("biasr", [128, OSH], F32, isOutput=False)
    kcol = nc.declare_dram_parameter("kcol", [128, 1], F32, isOutput=False)
    out = nc.declare_dram_parameter("out", [BN, OSH], BF16, isOutput=True)

    with tile.TileContext(nc) as tc:
        with (
            tc.tile_pool(name="consts", bufs=1) as constp,
            tc.tile_pool(name="xt", bufs=6) as xtp,
            tc.tile_pool(name="wt", bufs=6) as wp,
            tc.tile_pool(name="xct", bufs=3) as xctp,
            tc.tile_pool(name="work", bufs=2) as workp,
            tc.tile_pool(name="stg", bufs=6) as stgp,
            tc.tile_pool(name="strep", bufs=4) as strepp,
            tc.tile_pool(name="half", bufs=32) as halfp,
            tc.tile_pool(name="lut", bufs=1) as lutp,
            tc.tile_pool(name="outs", bufs=2) as outp,
            tc.tile_pool(name="ps", bufs=8, space="PSUM") as psp,
            tc.tile_pool(name="dram", bufs=1, space="DRAM") as dramp,
        ):
            # ---- constants -------------------------------------------------
            cbdh_sb = constp.tile([128, CK], BF16, tag="cbdh_sb")
            nc.sync.dma_start(out=cbdh_sb[:, :], in_=cbd_h.ap()[:, :])
            cbdl_sb = constp.tile([128, CK], BF16, tag="cbdl_sb")
            nc.sync.dma_start(out=cbdl_sb[:, :], in_=cbd_l.ap()[:, :])
            c2_sb = constp.tile([128, CK], F32, tag="c2_sb")
            nc.sync.dma_start(out=c2_sb[:, :], in_=c2r.ap()[:, :])
            iota_sb = constp.tile([128, 128], F32, tag="iota_sb")
            nc.sync.dma_start(out=iota_sb[:, :], in_=iotar.ap()[:, :])
            identb = constp.tile([128, 128], BF16, tag="identb")
            make_identity(nc, identb[:, :])
            identf = constp.tile([128, 128], F32, tag="identf")
            make_identity(nc, identf[:, :])

            # DRAM scratch for collectives
            kt_bounce1 = dramp.tile([NC // 2, TOK], BF16, tag="kt_bounce1")
            kt_bounce2 = dramp.tile([NC // 2, TOK], BF16, tag="kt_bounce2")
            kt_all1 = dramp.tile([N_CORES * NC // 2, TOK], BF16, tag="kt_all1")
            kt_all2 = dramp.tile([N_CORES * NC // 2, TOK], BF16, tag="kt_all2")
            mx_in = dramp.tile([1, 16], F32, tag="mx_in")
            mx_out = dramp.tile([1, 16], F32, tag="mx_out")

            # ---- phase A: lut = centroids @ weight (3-pass bf16 hi/lo) ----
            # Runs first so the absmax -> AllReduce(max) chain is issued as
            # early as possible (the collective queue is FIFO: the first
            # collective blocks the second until it completes).
            lut_sb = lutp.tile([128, NCHUNK * OSH], F32, tag="lut")
            mx8 = constp.tile([128, NCHUNK], F32, tag="mx8")
            for j in range(NCHUNK):
                ps_lut = psp.tile([128, OSH], F32, tag="ps")
                for mcol in range(4):
                    p = 4 * j + mcol
                    w_t = wp.tile([128, 2 * OSH], BF16, tag="wt")
                    nc.scalar.dma_start(
                        out=w_t[:, :],
                        in_=whl.ap()[128 * p:128 * (p + 1), :])
                    passes = [(cbdh_sb, 0), (cbdh_sb, OSH), (cbdl_sb, 0)]
                    for i, (cb, off) in enumerate(passes):
                        nc.tensor.matmul(
                            ps_lut[32 * mcol:32 * (mcol + 1), :],
                            lhsT=cb[:, 32 * p:32 * (p + 1)],
                            rhs=w_t[:, off:off + OSH],
                            start=(i == 0), stop=(i == 2),
                            tile_position=(0, 32 * mcol),
                        )
                nc.vector.tensor_copy(
                    out=lut_sb[:, OSH * j:OSH * (j + 1)], in_=ps_lut[:, :])
                nc.vector.tensor_reduce(
                    out=mx8[:, j:j + 1],
                    in_=lut_sb[:, OSH * j:OSH * (j + 1)],
                    axis=mybir.AxisListType.X, op=mybir.AluOpType.max,
                    apply_absolute_value=True,
                )

            # ---- phase B: global absmax -> AllReduce(max) -> scale ---------
            mxl = constp.tile([128, 1], F32, tag="mxl")
            nc.vector.tensor_reduce(
                out=mxl[:, :], in_=mx8[:, :], axis=mybir.AxisListType.X,
                op=mybir.AluOpType.max,
            )
            mxp = constp.tile([128, 1], F32, tag="mxp")
            nc.gpsimd.partition_all_reduce(
                mxp[:, :], mxl[:, :], channels=128,
                reduce_op=bass_isa.ReduceOp.max,
            )
            zrow = constp.tile([1, 16], F32, tag="zrow")
            nc.vector.memset(zrow[:, :], 0.0)
            nc.vector.tensor_copy(out=zrow[0:1, 0:1], in_=mxp[0:1, 0:1])
            nc.gpsimd.dma_start(out=mx_in[:, :], in_=zrow[:, :])
            nc.gpsimd.collective_compute(
                "AllReduce",
                mybir.AluOpType.max,
                replica_groups=[list(range(N_CORES))],
                ins=[mx_in.opt()],
                outs=[mx_out.opt()],
            )
            gmax_row = constp.tile([1, 16], F32, tag="gmax_row")
            nc.gpsimd.dma_start(out=gmax_row[:, :], in_=mx_out[:, :])
            gmax = constp.tile([128, 1], F32, tag="gmax")
            nc.gpsimd.partition_broadcast(gmax[:, :], gmax_row[0:1, 0:1])
            # s = gmax/127 and 1/s via reciprocal (DVE has no divide); the
            # <=2ulp drift is far inside the error budget.
            s_col = constp.tile([128, 1], F32, tag="s_col")
            nc.vector.tensor_scalar(
                out=s_col[:, :], in0=gmax[:, :],
                scalar1=float(np.float32(1.0) / np.float32(127.0)),
                scalar2=None, op0=mybir.AluOpType.mult,
            )
            rgmax = constp.tile([128, 1], F32, tag="rgmax")
            nc.vector.reciprocal(rgmax[:, :], gmax[:, :])
            inv_s = constp.tile([128, 1], F32, tag="inv_s")
            nc.vector.tensor_scalar(
                out=inv_s[:, :], in0=rgmax[:, :], scalar1=127.0, scalar2=None,
                op0=mybir.AluOpType.mult,
            )

            # ---- phase C: scores + incremental argmin ----------------------
            # The DVE executes its stream in order, so the argmin chain for
            # chunk j is emitted AFTER chunk j+1's PSUM evict: the evict is
            # never stuck behind argmin work and PE transposes stay fed.
            kminbig = workp.tile([128, NT * NC], F32, tag="kminbig", bufs=1)
            ps_trs = [None] * NCHUNK

            def argmin_chunk(j):
                ps_tr4 = ps_trs[j]
                ssl = workp.tile([128, TOK], F32, tag="ssl", name=f"ssl{j}")
                nc.vector.scalar_tensor_tensor(
                    out=ssl[:, :].rearrange("p (t f) -> p t f", f=128),
                    in0=ps_tr4[:, :].rearrange("p (t f) -> p t f", f=128),
                    in1=c2_sb[:, 128 * j:128 * (j + 1)].unsqueeze(1)
                        .broadcast_to([128, NT, 128]),
                    scalar=-2.0,
                    op0=mybir.AluOpType.mult,
                    op1=mybir.AluOpType.add,
                )
                ssl4 = ssl[:, :].rearrange("p (t c k) -> p t c k", k=K, c=8)
                m32 = workp.tile([128, NT * 8], F32, tag="m32", name=f"m32_{j}")
                nc.vector.tensor_reduce(
                    out=m32[:, :], in_=ssl4, axis=mybir.AxisListType.X,
                    op=mybir.AluOpType.min,
                )
                eq = workp.tile([128, TOK], F32, tag="eq", name=f"eq{j}")
                nc.vector.tensor_tensor(
                    out=eq[:, :].rearrange("p (t c k) -> p t c k", k=K, c=8),
                    in0=ssl4,
                    in1=m32[:, :].rearrange("p (t c) -> p t c", c=8)
                        .unsqueeze(3).broadcast_to([128, NT, 8, K]),
                    op=mybir.AluOpType.is_equal,
                )
                cand = workp.tile([128, TOK], F32, tag="cand", name=f"cand{j}")
                nc.vector.scalar_tensor_tensor(
                    out=cand[:, :].rearrange("p (t f) -> p t f", f=128),
                    in0=eq[:, :].rearrange("p (t f) -> p t f", f=128),
                    in1=iota_sb[:, :].unsqueeze(1)
                        .broadcast_to([128, NT, 128]),
                    scalar=-BIG,
                    op0=mybir.AluOpType.mult, op1=mybir.AluOpType.add,
                )
                nc.vector.tensor_reduce(
                    out=kminbig[:, :].rearrange(
                        "p (t c) -> p t c", c=NC)[:, :, 8 * j:8 * (j + 1)],
                    in_=cand[:, :].rearrange("p (t c k) -> p t c k", k=K, c=8),
                    axis=mybir.AxisListType.X, op=mybir.AluOpType.min,
                )

            def ship_half(half):
                # kmin [n, c-half] -> bf16 -> transpose -> AllGather.
                # Collective order (AR, AG1, AG2) matches input readiness so
                # the FIFO collective queue never head-of-line blocks.
                c0 = (NC // 2) * half
                kth = constp.tile([NC // 2, TOK], BF16, tag=f"kt_sb{half}",
                                  name=f"kt_sb{half}")
                for t in range(NT):
                    kminb = workp.tile([128, NC // 2], BF16, tag="kminb",
                                       name=f"kminb{half}_{t}")
                    nc.vector.tensor_scalar_add(
                        kminb[:, :].rearrange("p (c j) -> p c j", j=4),
                        kminbig[:, NC * t + c0:NC * t + c0 + NC // 2]
                            .rearrange("p (j c) -> p j c", c=8)
                            .transpose([0, 2, 1]),
                        BIG)
                    ps_kt = psp.tile([NC // 2, 128], BF16, tag="ps",
                                     name=f"ps_kt{half}_{t}")
                    nc.tensor.transpose(ps_kt[:, :], kminb[:, :],
                                        identb[:, :])
                    nc.vector.tensor_copy(
                        out=kth[:, 128 * t:128 * (t + 1)], in_=ps_kt[:, :])
                bnc = kt_bounce1 if half == 0 else kt_bounce2
                gat = kt_all1 if half == 0 else kt_all2
                nc.gpsimd.dma_start(out=bnc[:, :], in_=kth[:, :])
                nc.gpsimd.collective_compute(
                    "AllGather",
                    mybir.AluOpType.bypass,
                    replica_groups=[list(range(N_CORES))],
                    ins=[bnc.opt()],
                    outs=[gat.opt()],
                )

            for j in range(NCHUNK):
                ps_xct = psp.tile([128, TOK], F32, tag="ps", name=f"ps_xct{j}")
                for mcol in range(4):
                    p = 4 * j + mcol
                    xt_t = xtp.tile([128, 2 * TOK], BF16, tag="xt")
                    nc.sync.dma_start(out=xt_t[:, :],
                                      in_=xhl.ap()[128 * p:128 * (p + 1), :])
                    passes = [(cbdh_sb, 0), (cbdh_sb, TOK), (cbdl_sb, 0)]
                    for i, (cb, off) in enumerate(passes):
                        nc.tensor.matmul(
                            ps_xct[32 * mcol:32 * (mcol + 1), :],
                            lhsT=cb[:, 32 * p:32 * (p + 1)],
                            rhs=xt_t[:, off:off + TOK],
                            start=(i == 0), stop=(i == 2),
                            tile_position=(0, 32 * mcol),
                        )
                xct_sb = xctp.tile([128, TOK], F32, tag="xct")
                nc.vector.tensor_copy(out=xct_sb[:, :], in_=ps_xct[:, :])
                ps_tr4 = psp.tile([128, TOK], F32, tag="ps", name=f"ps_tr4_{j}")
                for t in range(NT):
                    nc.tensor.transpose(
                        ps_tr4[:, 128 * t:128 * (t + 1)],
                        xct_sb[:, 128 * t:128 * (t + 1)],
                        identf[:, :],
                    )
                ps_trs[j] = ps_tr4
                if j > 0:
                    argmin_chunk(j - 1)
                    ps_trs[j - 1] = None
                if j == 4:
                    ship_half(0)
            argmin_chunk(NCHUNK - 1)
            ship_half(1)

            # ---- phase 5: quantize-dequantize lut -> bf16 ------------------
            lutdq = lutp.tile([128, NCHUNK * OSH], BF16, tag="lutdq")
            for j in range(NCHUNK):
                qm = wp.tile([128, OSH], F32, tag="qm")
                nc.vector.tensor_scalar(
                    out=qm[:, :], in0=lut_sb[:, OSH * j:OSH * (j + 1)],
                    scalar1=inv_s[:, 0:1], scalar2=MAGIC,
                    op0=mybir.AluOpType.mult, op1=mybir.AluOpType.add,
                )
                nc.vector.tensor_scalar(
                    out=lutdq[:, OSH * j:OSH * (j + 1)], in0=qm[:, :],
                    scalar1=MAGIC, scalar2=s_col[:, 0:1],
                    op0=mybir.AluOpType.subtract, op1=mybir.AluOpType.mult,
                )

            bias_sb = constp.tile([128, OSH], F32, tag="bias_sb")
            nc.sync.dma_start(out=bias_sb[:, :], in_=biasr.ap()[:, :])
            kcol_sb = constp.tile([128, 1], F32, tag="kcol_sb")
            nc.sync.dma_start(out=kcol_sb[:, :], in_=kcol.ap()[:, :])

            # ---- phase 6: expand one-hots + gather matmul ------------------
            # Two half-contractions: chunks 0-3 right after AG1 (overlapping
            # AG2), chunks 4-7 after AG2; bias folded into the first half.
            # One 4D-AP replication DMA + one is_equal per (half, rank).
            half_tiles = {}
            for half in range(2):
                gat = kt_all1 if half == 0 else kt_all2
                for r in range(N_CORES):
                    rep = strepp.tile([128, 4 * TOK], BF16, tag="strep")
                    nc.scalar.dma_start(
                        out=rep[:, :],
                        in_=gat[32 * r:32 * (r + 1), :]
                            .rearrange("(c j) n -> c (j n)", j=4)
                            .unsqueeze(1).broadcast_to([8, K, 4 * TOK]),
                    )
                    stg_t = stgp.tile([128, 4 * TOK], BF16, tag="stg")
                    nc.vector.tensor_scalar(
                        out=stg_t[:, :], in0=rep[:, :],
                        scalar1=kcol_sb[:, 0:1], scalar2=None,
                        op0=mybir.AluOpType.is_equal,
                    )
                    o_sb = None
                    if half == 1:
                        o_sb = outp.tile([128, NT * OSH], BF16, tag="outsb")
                    for t in range(NT):
                        ps_o = psp.tile([128, OSH], F32, tag="ps")
                        for jj in range(NCHUNK // 2):
                            jglob = 4 * half + jj
                            nc.tensor.matmul(
                                ps_o[:, :],
                                lhsT=stg_t[:, TOK * jj + 128 * t:
                                           TOK * jj + 128 * (t + 1)],
                                rhs=lutdq[:, OSH * jglob:OSH * (jglob + 1)],
                                start=(jj == 0), stop=(jj == NCHUNK // 2 - 1),
                            )
                        if half == 0:
                            hs = halfp.tile([128, OSH], BF16, tag="hsum",
                                            name=f"hs{r}_{t}")
                            nc.vector.tensor_tensor(
                                out=hs[:, :], in0=ps_o[:, :], in1=bias_sb[:, :],
                                op=mybir.AluOpType.add,
                            )
                            half_tiles[(r, t)] = hs
                        else:
                            nc.vector.tensor_tensor(
                                out=o_sb[:, OSH * t:OSH * (t + 1)],
                                in0=ps_o[:, :], in1=half_tiles[(r, t)][:, :],
                                op=mybir.AluOpType.add,
                            )
                    if half == 1:
                        nc.sync.dma_start(
                            out=out.ap()[TOK * r:TOK * (r + 1), :]
                                .rearrange("(t p) o -> p t o", p=128),
                            in_=o_sb[:, :].rearrange("p (t o) -> p t o", o=OSH),
                        )

    nc.compile()
    return nc


def _prep_inputs(x, centroids, weight, bias):
    import ml_dtypes

    x = np.ascontiguousarray(np.asarray(x, dtype=np.float32)).reshape(BN, IN_F)
    cent = np.asarray(centroids, dtype=np.float32)
    w = np.asarray(weight, dtype=np.float32)
    bias = np.asarray(bias, dtype=np.float32)

    c2 = (cent ** 2).sum(axis=-1).reshape(CK)  # [1024] flat (c,k)
    c2r = np.ascontiguousarray(np.broadcast_to(c2, (128, CK)))
    iota = np.tile(np.arange(K, dtype=np.float32), 8)
    iotar = np.ascontiguousarray(np.broadcast_to(iota, (128, 128)))
    cbd = np.zeros((128, CK), np.float32)
    for p in range(NPAIR):
        cbd[0:SUBV, 32 * p:32 * p + K] = cent[2 * p].T
        cbd[SUBV:128, 32 * p + K:32 * p + 2 * K] = cent[2 * p + 1].T
    cbd_h = cbd.astype(ml_dtypes.bfloat16)
    cbd_l = (cbd - cbd_h.astype(np.float32)).astype(ml_dtypes.bfloat16)
    kcol = np.ascontiguousarray(
        (np.arange(128, dtype=np.float32) % K).reshape(128, 1))

    in_maps = []
    for r in range(N_CORES):
        xT_r = np.ascontiguousarray(x[TOK * r:TOK * (r + 1)].T)
        xh_r = xT_r.astype(ml_dtypes.bfloat16)
        xl_r = (xT_r - xh_r.astype(np.float32)).astype(ml_dtypes.bfloat16)
        xhl_r = np.ascontiguousarray(np.concatenate([xh_r, xl_r], axis=1))
        w_r = np.ascontiguousarray(w[:, :, OSH * r:OSH * (r + 1)]).reshape(
            IN_F, OSH)
        w_h = w_r.astype(ml_dtypes.bfloat16)
        w_l = (w_r - w_h.astype(np.float32)).astype(ml_dtypes.bfloat16)
        whl_r = np.ascontiguousarray(np.concatenate([w_h, w_l], axis=1))
        bias_r = np.ascontiguousarray(
            np.broadcast_to(bias[OSH * r:OSH * (r + 1)], (128, OSH)))
        in_maps.append({
            "xhl": xhl_r, "whl": whl_r, "cbd_h": cbd_h, "cbd_l": cbd_l,
            "c2r": c2r, "iotar": iotar, "biasr": bias_r, "kcol": kcol,
        })
    return in_maps


def kernel(x, centroids, weight, inverse_temperature_logit, bias,
           **_unused) -> np.ndarray:
    if "nc" not in _CACHE:
        _CACHE["nc"] = _build()
    nc = _CACHE["nc"]
    in_maps = _prep_inputs(x, centroids, weight, bias)
    res = run_bass_kernel_spmd(nc, in_maps, core_ids=list(range(N_CORES)))
    out = np.concatenate(
        [res.results[r]["out"].astype(np.float32) for r in range(N_CORES)],
        axis=1)
    return out.reshape(2, BN // 2, OUT_F)


# revision 35
# speedup vs baseline: 1.0937x; 1.0261x over previous
"""AMMLinear (VQ codebook) forward on 8 TRN2 NeuronCores.

The straight-through estimator makes the forward VALUE exactly
    out[n, o] = sum_c lut_dq[c, argmin_k dist(x_cn, cent_ck), o] + bias[o]
with lut = centroids @ weight (per codebook) and lut_dq a global-scale int8
quantize-dequantize of lut.  The softmax/attention path only shapes gradients.

Sharding: tokens (BN=4096 -> 512/core) for the score/argmin phase, output
features (4096 -> 512/core) for the lut/gather phase.  One AllGather moves the
bf16 argmin indices (64KB/core), one AllReduce-max the quantization scale.
Every core then expands all 4096 tokens' one-hot codes locally (replication
DMA + is_equal) and computes its o-shard of the gather matmul.
Output is assembled host-side by concatenating the per-core o-shards.

Numerics: scores and lut are computed as 3-pass bf16 hi/lo matmuls
(a*b ~ ah*bh + ah*bl + al*bh, fp32 PSUM accumulate, ~2^-18/product), the
gather matmul in bf16 (exact one-hots, bf16-rounded lut_dq), output in bf16.
Measured end-to-end rel err ~5e-3 against the fp32 reference.
"""

import numpy as np

import concourse.bass as bass
import concourse.mybir as mybir
import concourse.tile as tile
import concourse.bass_isa as bass_isa
from concourse import bacc
from concourse.bass_utils import run_bass_kernel_spmd
from concourse.masks import make_identity

F32 = mybir.dt.float32
BF16 = mybir.dt.bfloat16

N_CORES = 8
NC, K, IN_F, OUT_F = 64, 16, 4096, 4096
SUBV = IN_F // NC          # 64
BN = 4096                  # 2*2048 tokens
TOK = BN // N_CORES        # 512 tokens per core
NT = TOK // 128            # 4 token tiles per core
NPAIR = NC // 2            # 32 codebook pairs
CK = NC * K                # 1024 (codebook,centroid) flat index
NCHUNK = CK // 128         # 8 contraction chunks
OSH = OUT_F // N_CORES     # 512 out features per core
MAGIC = 12582912.0         # 1.5 * 2**23: fp32 round-to-nearest-even trick
BIG = 1024.0

_CACHE = {}


def _build():
    nc = bacc.Bacc("TRN2", target_bir_lowering=False, debug=False,
                   num_devices=N_CORES)

    # xhl[:, 0:TOK] = bf16 hi of x^T shard, [:, TOK:2*TOK] = bf16 lo
    xhl = nc.declare_dram_parameter("xhl", [IN_F, 2 * TOK], BF16,
                                    isOutput=False)
    # whl[:, 0:OSH] = bf16 hi of weight o-shard, [:, OSH:2*OSH] = lo
    whl = nc.declare_dram_parameter("whl", [IN_F, 2 * OSH], BF16,
                                    isOutput=False)
    cbd_h = nc.declare_dram_parameter("cbd_h", [128, CK], BF16, isOutput=False)
    cbd_l = nc.declare_dram_parameter("cbd_l", [128, CK], BF16, isOutput=False)
    c2r = nc.declare_dram_parameter("c2r", [128, CK], F32, isOutput=False)
    iotar = nc.declare_dram_parameter("iotar", [128, 128], F32, isOutput=False)
    biasr = nc.declare_dram_parameter("biasr", [128, OSH], F32, isOutput=False)
    kcol = nc.declare_dram_parameter("kcol", [128, 1], F32, isOutput=False)
    out = nc.declare_dram_parameter("out", [BN, OSH], BF16, isOutput=True)

    with tile.TileContext(nc) as tc:
        with (
            tc.tile_pool(name="consts", bufs=1) as constp,
            tc.tile_pool(name="xt", bufs=6) as xtp,
            tc.tile_pool(name="wt", bufs=6) as wp,
            tc.tile_pool(name="xct", bufs=3) as xctp,
            tc.tile_pool(name="work", bufs=2) as workp,
            tc.tile_pool(name="stg", bufs=8) as stgp,
            tc.tile_pool(name="strep", bufs=6) as strepp,
            tc.tile_pool(name="half", bufs=32) as halfp,
            tc.tile_pool(name="lut", bufs=1) as lutp,
            tc.tile_pool(name="outs", bufs=2) as outp,
            tc.tile_pool(name="ps", bufs=8, space="PSUM") as psp,
            tc.tile_pool(name="dram", bufs=1, space="DRAM") as dramp,
        ):
            # ---- constants -------------------------------------------------
            cbdh_sb = constp.tile([128, CK], BF16, tag="cbdh_sb")
            nc.sync.dma_start(out=cbdh_sb[:, :], in_=cbd_h.ap()[:, :])
            cbdl_sb = constp.tile([128, CK], BF16, tag="cbdl_sb")
            nc.sync.dma_start(out=cbdl_sb[:, :], in_=cbd_l.ap()[:, :])
            c2_sb = constp.tile([128, CK], F32, tag="c2_sb")
            nc.sync.dma_start(out=c2_sb[:, :], in_=c2r.ap()[:, :])
            iota_sb = constp.tile([128, 128], F32, tag="iota_sb")
            nc.sync.dma_start(out=iota_sb[:, :], in_=iotar.ap()[:, :])
            identb = constp.tile([128, 128], BF16, tag="identb")
            make_identity(nc, identb[:, :])
            identf = constp.tile([128, 128], F32, tag="identf")
            make_identity(nc, identf[:, :])

            # DRAM scratch for collectives
            kt_bounce1 = dramp.tile([NC // 2, TOK], BF16, tag="kt_bounce1")
            kt_bounce2 = dramp.tile([NC // 2, TOK], BF16, tag="kt_bounce2")
            kt_all1 = dramp.tile([N_CORES * NC // 2, TOK], BF16, tag="kt_all1")
            kt_all2 = dramp.tile([N_CORES * NC // 2, TOK], BF16, tag="kt_all2")
            mx_in = dramp.tile([1, 16], F32, tag="mx_in")
            mx_out = dramp.tile([1, 16], F32, tag="mx_out")

            # ---- phase A: lut = centroids @ weight (3-pass bf16 hi/lo) ----
            # Runs first so the absmax -> AllReduce(max) chain is issued as
            # early as possible (the collective queue is FIFO: the first
            # collective blocks the second until it completes).
            lut_sb = lutp.tile([128, NCHUNK * OSH], F32, tag="lut")
            mx8 = constp.tile([128, NCHUNK], F32, tag="mx8")
            for j in range(NCHUNK):
                ps_lut = psp.tile([128, OSH], F32, tag="ps")
                for mcol in range(4):
                    p = 4 * j + mcol
                    w_t = wp.tile([128, 2 * OSH], BF16, tag="wt")
                    nc.scalar.dma_start(
                        out=w_t[:, :],
                        in_=whl.ap()[128 * p:128 * (p + 1), :])
                    passes = [(cbdh_sb, 0), (cbdh_sb, OSH), (cbdl_sb, 0)]
                    for i, (cb, off) in enumerate(passes):
                        nc.tensor.matmul(
                            ps_lut[32 * mcol:32 * (mcol + 1), :],
                            lhsT=cb[:, 32 * p:32 * (p + 1)],
                            rhs=w_t[:, off:off + OSH],
                            start=(i == 0), stop=(i == 2),
                            tile_position=(0, 32 * mcol),
                        )
                nc.vector.tensor_copy(
                    out=lut_sb[:, OSH * j:OSH * (j + 1)], in_=ps_lut[:, :])
                nc.vector.tensor_reduce(
                    out=mx8[:, j:j + 1],
                    in_=lut_sb[:, OSH * j:OSH * (j + 1)],
                    axis=mybir.AxisListType.X, op=mybir.AluOpType.max,
                    apply_absolute_value=True,
                )

            # ---- phase B: global absmax -> AllReduce(max) -> scale ---------
            mxl = constp.tile([128, 1], F32, tag="mxl")
            nc.vector.tensor_reduce(
                out=mxl[:, :], in_=mx8[:, :], axis=mybir.AxisListType.X,
                op=mybir.AluOpType.max,
            )
            mxp = constp.tile([128, 1], F32, tag="mxp")
            nc.gpsimd.partition_all_reduce(
                mxp[:, :], mxl[:, :], channels=128,
                reduce_op=bass_isa.ReduceOp.max,
            )
            zrow = constp.tile([1, 16], F32, tag="zrow")
            nc.vector.memset(zrow[:, :], 0.0)
            nc.vector.tensor_copy(out=zrow[0:1, 0:1], in_=mxp[0:1, 0:1])
            nc.gpsimd.dma_start(out=mx_in[:, :], in_=zrow[:, :])
            nc.gpsimd.collective_compute(
                "AllReduce",
                mybir.AluOpType.max,
                replica_groups=[list(range(N_CORES))],
                ins=[mx_in.opt()],
                outs=[mx_out.opt()],
            )

            # ---- phase C: scores + incremental argmin ----------------------
            # The DVE executes its stream in order, so the argmin chain for
            # chunk j is emitted AFTER chunk j+1's PSUM evict: the evict is
            # never stuck behind argmin work and PE transposes stay fed.
            kminbig = workp.tile([128, NT * NC], F32, tag="kminbig", bufs=1)
            ps_trs = [None] * NCHUNK

            def argmin_chunk(j):
                ps_tr4 = ps_trs[j]
                ssl = workp.tile([128, TOK], F32, tag="ssl", name=f"ssl{j}")
                nc.vector.scalar_tensor_tensor(
                    out=ssl[:, :].rearrange("p (t f) -> p t f", f=128),
                    in0=ps_tr4[:, :].rearrange("p (t f) -> p t f", f=128),
                    in1=c2_sb[:, 128 * j:128 * (j + 1)].unsqueeze(1)
                        .broadcast_to([128, NT, 128]),
                    scalar=-2.0,
                    op0=mybir.AluOpType.mult,
                    op1=mybir.AluOpType.add,
                )
                ssl4 = ssl[:, :].rearrange("p (t c k) -> p t c k", k=K, c=8)
                m32 = workp.tile([128, NT * 8], F32, tag="m32", name=f"m32_{j}")
                nc.vector.tensor_reduce(
                    out=m32[:, :], in_=ssl4, axis=mybir.AxisListType.X,
                    op=mybir.AluOpType.min,
                )
                eq = workp.tile([128, TOK], F32, tag="eq", name=f"eq{j}")
                nc.vector.tensor_tensor(
                    out=eq[:, :].rearrange("p (t c k) -> p t c k", k=K, c=8),
                    in0=ssl4,
                    in1=m32[:, :].rearrange("p (t c) -> p t c", c=8)
                        .unsqueeze(3).broadcast_to([128, NT, 8, K]),
                    op=mybir.AluOpType.is_equal,
                )
                cand = workp.tile([128, TOK], F32, tag="cand", name=f"cand{j}")
                nc.vector.scalar_tensor_tensor(
                    out=cand[:, :].rearrange("p (t f) -> p t f", f=128),
                    in0=eq[:, :].rearrange("p (t f) -> p t f", f=128),
                    in1=iota_sb[:, :].unsqueeze(1)
                        .broadcast_to([128, NT, 128]),
                    scalar=-BIG,
                    op0=mybir.AluOpType.mult, op1=mybir.AluOpType.add,
                )
                nc.vector.tensor_reduce(
                    out=kminbig[:, :].rearrange(
                        "p (t c) -> p t c", c=NC)[:, :, 8 * j:8 * (j + 1)],
                    in_=cand[:, :].rearrange("p (t c k) -> p t c k", k=K, c=8),
                    axis=mybir.AxisListType.X, op=mybir.AluOpType.min,
                )

            def ship_half(half):
                # kmin [n, c-half] -> bf16 -> transpose -> AllGather.
                # Collective order (AR, AG1, AG2) matches input readiness so
                # the FIFO collective queue never head-of-line blocks.
                c0 = (NC // 2) * half
                kth = constp.tile([NC // 2, TOK], BF16, tag=f"kt_sb{half}",
                                  name=f"kt_sb{half}")
                for t in range(NT):
                    kminb = workp.tile([128, NC // 2], BF16, tag="kminb",
                                       name=f"kminb{half}_{t}")
                    nc.vector.tensor_scalar_add(
                        kminb[:, :].rearrange("p (c j) -> p c j", j=4),
                        kminbig[:, NC * t + c0:NC * t + c0 + NC // 2]
                            .rearrange("p (j c) -> p j c", c=8)
                            .transpose([0, 2, 1]),
                        BIG)
                    ps_kt = psp.tile([NC // 2, 128], BF16, tag="ps",
                                     name=f"ps_kt{half}_{t}")
                    nc.tensor.transpose(ps_kt[:, :], kminb[:, :],
                                        identb[:, :])
                    nc.vector.tensor_copy(
                        out=kth[:, 128 * t:128 * (t + 1)], in_=ps_kt[:, :])
                bnc = kt_bounce1 if half == 0 else kt_bounce2
                gat = kt_all1 if half == 0 else kt_all2
                nc.gpsimd.dma_start(out=bnc[:, :], in_=kth[:, :])
                nc.gpsimd.collective_compute(
                    "AllGather",
                    mybir.AluOpType.bypass,
                    replica_groups=[list(range(N_CORES))],
                    ins=[bnc.opt()],
                    outs=[gat.opt()],
                )

            for j in range(NCHUNK):
                ps_xct = psp.tile([128, TOK], F32, tag="ps", name=f"ps_xct{j}")
                for mcol in range(4):
                    p = 4 * j + mcol
                    xt_t = xtp.tile([128, 2 * TOK], BF16, tag="xt")
                    nc.sync.dma_start(out=xt_t[:, :],
                                      in_=xhl.ap()[128 * p:128 * (p + 1), :])
                    passes = [(cbdh_sb, 0), (cbdh_sb, TOK), (cbdl_sb, 0)]
                    for i, (cb, off) in enumerate(passes):
                        nc.tensor.matmul(
                            ps_xct[32 * mcol:32 * (mcol + 1), :],
                            lhsT=cb[:, 32 * p:32 * (p + 1)],
                            rhs=xt_t[:, off:off + TOK],
                            start=(i == 0), stop=(i == 2),
                            tile_position=(0, 32 * mcol),
                        )
                xct_sb = xctp.tile([128, TOK], F32, tag="xct")
                nc.vector.tensor_copy(out=xct_sb[:, :], in_=ps_xct[:, :])
                ps_tr4 = psp.tile([128, TOK], F32, tag="ps", name=f"ps_tr4_{j}")
                for t in range(NT):
                    nc.tensor.transpose(
                        ps_tr4[:, 128 * t:128 * (t + 1)],
                        xct_sb[:, 128 * t:128 * (t + 1)],
                        identf[:, :],
                    )
                ps_trs[j] = ps_tr4
                if j > 0:
                    argmin_chunk(j - 1)
                    ps_trs[j - 1] = None
                if j == 4:
                    ship_half(0)
            argmin_chunk(NCHUNK - 1)
            ship_half(1)

            # gmax recovery AFTER the AG triggers: the gpsimd stream must not
            # stall on the AllReduce wait before issuing the AllGathers.
            gmax_row = constp.tile([1, 16], F32, tag="gmax_row")
            nc.gpsimd.dma_start(out=gmax_row[:, :], in_=mx_out[:, :])
            gmax = constp.tile([128, 1], F32, tag="gmax")
            nc.gpsimd.partition_broadcast(gmax[:, :], gmax_row[0:1, 0:1])
            # s = gmax/127 and 1/s via reciprocal (DVE has no divide); the
            # <=2ulp drift is far inside the error budget.
            s_col = constp.tile([128, 1], F32, tag="s_col")
            nc.vector.tensor_scalar(
                out=s_col[:, :], in0=gmax[:, :],
                scalar1=float(np.float32(1.0) / np.float32(127.0)),
                scalar2=None, op0=mybir.AluOpType.mult,
            )
            rgmax = constp.tile([128, 1], F32, tag="rgmax")
            nc.vector.reciprocal(rgmax[:, :], gmax[:, :])
            inv_s = constp.tile([128, 1], F32, tag="inv_s")
            nc.vector.tensor_scalar(
                out=inv_s[:, :], in0=rgmax[:, :], scalar1=127.0, scalar2=None,
                op0=mybir.AluOpType.mult,
            )

            # ---- phase 5: quantize-dequantize lut -> bf16 ------------------
            lutdq = lutp.tile([128, NCHUNK * OSH], BF16, tag="lutdq")
            for j in range(NCHUNK):
                qm = wp.tile([128, OSH], F32, tag="qm")
                nc.vector.tensor_scalar(
                    out=qm[:, :], in0=lut_sb[:, OSH * j:OSH * (j + 1)],
                    scalar1=inv_s[:, 0:1], scalar2=MAGIC,
                    op0=mybir.AluOpType.mult, op1=mybir.AluOpType.add,
                )
                nc.vector.tensor_scalar(
                    out=lutdq[:, OSH * j:OSH * (j + 1)], in0=qm[:, :],
                    scalar1=MAGIC, scalar2=s_col[:, 0:1],
                    op0=mybir.AluOpType.subtract, op1=mybir.AluOpType.mult,
                )

            bias_sb = constp.tile([128, OSH], F32, tag="bias_sb")
            nc.sync.dma_start(out=bias_sb[:, :], in_=biasr.ap()[:, :])
            kcol_sb = constp.tile([128, 1], F32, tag="kcol_sb")
            nc.sync.dma_start(out=kcol_sb[:, :], in_=kcol.ap()[:, :])

            # ---- phase 6: expand one-hots + gather matmul ------------------
            # Two half-contractions: chunks 0-3 right after AG1 (overlapping
            # AG2), chunks 4-7 after AG2; bias folded into the first half.
            # One 4D-AP replication DMA + one is_equal per (half, rank).
            half_tiles = {}
            for half in range(2):
                gat = kt_all1 if half == 0 else kt_all2
                for r in range(N_CORES):
                    rep = strepp.tile([128, 4 * TOK], BF16, tag="strep")
                    nc.scalar.dma_start(
                        out=rep[:, :],
                        in_=gat[32 * r:32 * (r + 1), :]
                            .rearrange("(c j) n -> c (j n)", j=4)
                            .unsqueeze(1).broadcast_to([8, K, 4 * TOK]),
                    )
                    stg_t = stgp.tile([128, 4 * TOK], BF16, tag="stg")
                    nc.vector.tensor_scalar(
                        out=stg_t[:, :], in0=rep[:, :],
                        scalar1=kcol_sb[:, 0:1], scalar2=None,
                        op0=mybir.AluOpType.is_equal,
                    )
                    o_sb = None
                    if half == 1:
                        o_sb = outp.tile([128, NT * OSH], BF16, tag="outsb")
                    for t in range(NT):
                        ps_o = psp.tile([128, OSH], F32, tag="ps")
                        for jj in range(NCHUNK // 2):
                            jglob = 4 * half + jj
                            nc.tensor.matmul(
                                ps_o[:, :],
                                lhsT=stg_t[:, TOK * jj + 128 * t:
                                           TOK * jj + 128 * (t + 1)],
                                rhs=lutdq[:, OSH * jglob:OSH * (jglob + 1)],
                                start=(jj == 0), stop=(jj == NCHUNK // 2 - 1),
                            )
                        if half == 0:
                            hs = halfp.tile([128, OSH], BF16, tag="hsum",
                                            name=f"hs{r}_{t}")
                            nc.vector.tensor_tensor(
                                out=hs[:, :], in0=ps_o[:, :], in1=bias_sb[:, :],
                                op=mybir.AluOpType.add,
                            )
                            half_tiles[(r, t)] = hs
                        else:
                            nc.vector.tensor_tensor(
                                out=o_sb[:, OSH * t:OSH * (t + 1)],
                                in0=ps_o[:, :], in1=half_tiles[(r, t)][:, :],
                                op=mybir.AluOpType.add,
                            )
                    if half == 1:
                        nc.sync.dma_start(
                            out=out.ap()[TOK * r:TOK * (r + 1), :]
                                .rearrange("(t p) o -> p t o", p=128),
                            in_=o_sb[:, :].rearrange("p (t o) -> p t o", o=OSH),
                        )

    nc.compile()
    return nc


def _prep_inputs(x, centroids, weight, bias):
    import ml_dtypes

    x = np.ascontiguousarray(np.asarray(x, dtype=np.float32)).reshape(BN, IN_F)
    cent = np.asarray(centroids, dtype=np.float32)
    w = np.asarray(weight, dtype=np.float32)
    bias = np.asarray(bias, dtype=np.float32)

    c2 = (cent ** 2).sum(axis=-1).reshape(CK)  # [1024] flat (c,k)
    c2r = np.ascontiguousarray(np.broadcast_to(c2, (128, CK)))
    iota = np.tile(np.arange(K, dtype=np.float32), 8)
    iotar = np.ascontiguousarray(np.broadcast_to(iota, (128, 128)))
    cbd = np.zeros((128, CK), np.float32)
    for p in range(NPAIR):
        cbd[0:SUBV, 32 * p:32 * p + K] = cent[2 * p].T
        cbd[SUBV:128, 32 * p + K:32 * p + 2 * K] = cent[2 * p + 1].T
    cbd_h = cbd.astype(ml_dtypes.bfloat16)
    cbd_l = (cbd - cbd_h.astype(np.float32)).astype(ml_dtypes.bfloat16)
    kcol = np.ascontiguousarray(
        (np.arange(128, dtype=np.float32) % K).reshape(128, 1))

    in_maps = []
    for r in range(N_CORES):
        xT_r = np.ascontiguousarray(x[TOK * r:TOK * (r + 1)].T)
        xh_r = xT_r.astype(ml_dtypes.bfloat16)
        xl_r = (xT_r - xh_r.astype(np.float32)).astype(ml_dtypes.bfloat16)
        xhl_r = np.ascontiguousarray(np.concatenate([xh_r, xl_r], axis=1))
        w_r = np.ascontiguousarray(w[:, :, OSH * r:OSH * (r + 1)]).reshape(
            IN_F, OSH)
        w_h = w_r.astype(ml_dtypes.bfloat16)
        w_l = (w_r - w_h.astype(np.float32)).astype(ml_dtypes.bfloat16)
        whl_r = np.ascontiguousarray(np.concatenate([w_h, w_l], axis=1))
        bias_r = np.ascontiguousarray(
            np.broadcast_to(bias[OSH * r:OSH * (r + 1)], (128, OSH)))
        in_maps.append({
            "xhl": xhl_r, "whl": whl_r, "cbd_h": cbd_h, "cbd_l": cbd_l,
            "c2r": c2r, "iotar": iotar, "biasr": bias_r, "kcol": kcol,
        })
    return in_maps


def kernel(x, centroids, weight, inverse_temperature_logit, bias,
           **_unused) -> np.ndarray:
    if "nc" not in _CACHE:
        _CACHE["nc"] = _build()
    nc = _CACHE["nc"]
    in_maps = _prep_inputs(x, centroids, weight, bias)
    res = run_bass_kernel_spmd(nc, in_maps, core_ids=list(range(N_CORES)))
    out = np.concatenate(
        [res.results[r]["out"].astype(np.float32) for r in range(N_CORES)],
        axis=1)
    return out.reshape(2, BN // 2, OUT_F)


# revision 36
# speedup vs baseline: 1.1203x; 1.0243x over previous
"""AMMLinear (VQ codebook) forward on 8 TRN2 NeuronCores.

The straight-through estimator makes the forward VALUE exactly
    out[n, o] = sum_c lut_dq[c, argmin_k dist(x_cn, cent_ck), o] + bias[o]
with lut = centroids @ weight (per codebook) and lut_dq a global-scale int8
quantize-dequantize of lut.  The softmax/attention path only shapes gradients.

Sharding: tokens (BN=4096 -> 512/core) for the score/argmin phase, output
features (4096 -> 512/core) for the lut/gather phase.  One AllGather moves the
bf16 argmin indices (64KB/core), one AllReduce-max the quantization scale.
Every core then expands all 4096 tokens' one-hot codes locally (replication
DMA + is_equal) and computes its o-shard of the gather matmul.
Output is assembled host-side by concatenating the per-core o-shards.

Numerics: scores and lut are computed as 3-pass bf16 hi/lo matmuls
(a*b ~ ah*bh + ah*bl + al*bh, fp32 PSUM accumulate, ~2^-18/product), the
gather matmul in bf16 (exact one-hots, bf16-rounded lut_dq), output in bf16.
Measured end-to-end rel err ~5e-3 against the fp32 reference.
"""

import numpy as np

import concourse.bass as bass
import concourse.mybir as mybir
import concourse.tile as tile
import concourse.bass_isa as bass_isa
from concourse import bacc
from concourse.bass_utils import run_bass_kernel_spmd
from concourse.masks import make_identity

F32 = mybir.dt.float32
BF16 = mybir.dt.bfloat16

N_CORES = 8
NC, K, IN_F, OUT_F = 64, 16, 4096, 4096
SUBV = IN_F // NC          # 64
BN = 4096                  # 2*2048 tokens
TOK = BN // N_CORES        # 512 tokens per core
NT = TOK // 128            # 4 token tiles per core
NPAIR = NC // 2            # 32 codebook pairs
CK = NC * K                # 1024 (codebook,centroid) flat index
NCHUNK = CK // 128         # 8 contraction chunks
OSH = OUT_F // N_CORES     # 512 out features per core
MAGIC = 12582912.0         # 1.5 * 2**23: fp32 round-to-nearest-even trick
BIG = 1024.0

_CACHE = {}


def _build():
    nc = bacc.Bacc("TRN2", target_bir_lowering=False, debug=False,
                   num_devices=N_CORES)

    # xhl[:, 0:TOK] = bf16 hi of x^T shard, [:, TOK:2*TOK] = bf16 lo
    xhl = nc.declare_dram_parameter("xhl", [IN_F, 2 * TOK], BF16,
                                    isOutput=False)
    # w o-shard in fp16 (single-pass lut: ~2^-11/product, inside budget)
    wf16 = nc.declare_dram_parameter("wf16", [IN_F, OSH], mybir.dt.float16,
                                     isOutput=False)
    cbdf16 = nc.declare_dram_parameter("cbdf16", [128, CK], mybir.dt.float16,
                                       isOutput=False)
    cbd_h = nc.declare_dram_parameter("cbd_h", [128, CK], BF16, isOutput=False)
    cbd_l = nc.declare_dram_parameter("cbd_l", [128, CK], BF16, isOutput=False)
    c2r = nc.declare_dram_parameter("c2r", [128, CK], F32, isOutput=False)
    iotar = nc.declare_dram_parameter("iotar", [128, 128], F32, isOutput=False)
    biasr = nc.declare_dram_parameter("biasr", [128, OSH], F32, isOutput=False)
    kcol = nc.declare_dram_parameter("kcol", [128, 1], F32, isOutput=False)
    out = nc.declare_dram_parameter("out", [BN, OSH], BF16, isOutput=True)

    with tile.TileContext(nc) as tc:
        with (
            tc.tile_pool(name="consts", bufs=1) as constp,
            tc.tile_pool(name="xt", bufs=6) as xtp,
            tc.tile_pool(name="wt", bufs=6) as wp,
            tc.tile_pool(name="xct", bufs=3) as xctp,
            tc.tile_pool(name="work", bufs=2) as workp,
            tc.tile_pool(name="stg", bufs=8) as stgp,
            tc.tile_pool(name="strep", bufs=6) as strepp,
            tc.tile_pool(name="half", bufs=32) as halfp,
            tc.tile_pool(name="lut", bufs=1) as lutp,
            tc.tile_pool(name="outs", bufs=2) as outp,
            tc.tile_pool(name="ps", bufs=8, space="PSUM") as psp,
            tc.tile_pool(name="dram", bufs=1, space="DRAM") as dramp,
        ):
            # ---- constants -------------------------------------------------
            cbdf_sb = constp.tile([128, CK], mybir.dt.float16, tag="cbdf_sb")
            nc.sync.dma_start(out=cbdf_sb[:, :], in_=cbdf16.ap()[:, :])
            cbdh_sb = constp.tile([128, CK], BF16, tag="cbdh_sb")
            nc.sync.dma_start(out=cbdh_sb[:, :], in_=cbd_h.ap()[:, :])
            cbdl_sb = constp.tile([128, CK], BF16, tag="cbdl_sb")
            nc.sync.dma_start(out=cbdl_sb[:, :], in_=cbd_l.ap()[:, :])
            c2_sb = constp.tile([128, CK], F32, tag="c2_sb")
            nc.sync.dma_start(out=c2_sb[:, :], in_=c2r.ap()[:, :])
            iota_sb = constp.tile([128, 128], F32, tag="iota_sb")
            nc.sync.dma_start(out=iota_sb[:, :], in_=iotar.ap()[:, :])
            identb = constp.tile([128, 128], BF16, tag="identb")
            make_identity(nc, identb[:, :])
            identf = constp.tile([128, 128], F32, tag="identf")
            make_identity(nc, identf[:, :])

            # DRAM scratch for collectives
            kt_bounce1 = dramp.tile([NC // 2, TOK], BF16, tag="kt_bounce1")
            kt_bounce2 = dramp.tile([NC // 2, TOK], BF16, tag="kt_bounce2")
            kt_all1 = dramp.tile([N_CORES * NC // 2, TOK], BF16, tag="kt_all1")
            kt_all2 = dramp.tile([N_CORES * NC // 2, TOK], BF16, tag="kt_all2")
            mx_in = dramp.tile([1, 16], F32, tag="mx_in")
            mx_out = dramp.tile([1, 16], F32, tag="mx_out")

            # ---- phase A: lut = centroids @ weight (3-pass bf16 hi/lo) ----
            # Runs first so the absmax -> AllReduce(max) chain is issued as
            # early as possible (the collective queue is FIFO: the first
            # collective blocks the second until it completes).
            lut_sb = lutp.tile([128, NCHUNK * OSH], F32, tag="lut")
            mx8 = constp.tile([128, NCHUNK], F32, tag="mx8")
            for j in range(NCHUNK):
                ps_lut = psp.tile([128, OSH], F32, tag="ps")
                for mcol in range(4):
                    p = 4 * j + mcol
                    w_t = wp.tile([128, OSH], mybir.dt.float16, tag="wt")
                    nc.scalar.dma_start(
                        out=w_t[:, :],
                        in_=wf16.ap()[128 * p:128 * (p + 1), :])
                    nc.tensor.matmul(
                        ps_lut[32 * mcol:32 * (mcol + 1), :],
                        lhsT=cbdf_sb[:, 32 * p:32 * (p + 1)],
                        rhs=w_t[:, :],
                        start=True, stop=True,
                        tile_position=(0, 32 * mcol),
                    )
                nc.vector.tensor_copy(
                    out=lut_sb[:, OSH * j:OSH * (j + 1)], in_=ps_lut[:, :])
                nc.vector.tensor_reduce(
                    out=mx8[:, j:j + 1],
                    in_=lut_sb[:, OSH * j:OSH * (j + 1)],
                    axis=mybir.AxisListType.X, op=mybir.AluOpType.max,
                    apply_absolute_value=True,
                )

            # ---- phase B: global absmax -> AllReduce(max) -> scale ---------
            mxl = constp.tile([128, 1], F32, tag="mxl")
            nc.vector.tensor_reduce(
                out=mxl[:, :], in_=mx8[:, :], axis=mybir.AxisListType.X,
                op=mybir.AluOpType.max,
            )
            mxp = constp.tile([128, 1], F32, tag="mxp")
            nc.gpsimd.partition_all_reduce(
                mxp[:, :], mxl[:, :], channels=128,
                reduce_op=bass_isa.ReduceOp.max,
            )
            zrow = constp.tile([1, 16], F32, tag="zrow")
            nc.vector.memset(zrow[:, :], 0.0)
            nc.vector.tensor_copy(out=zrow[0:1, 0:1], in_=mxp[0:1, 0:1])
            nc.gpsimd.dma_start(out=mx_in[:, :], in_=zrow[:, :])
            nc.gpsimd.collective_compute(
                "AllReduce",
                mybir.AluOpType.max,
                replica_groups=[list(range(N_CORES))],
                ins=[mx_in.opt()],
                outs=[mx_out.opt()],
            )

            # ---- phase C: scores + incremental argmin ----------------------
            # The DVE executes its stream in order, so the argmin chain for
            # chunk j is emitted AFTER chunk j+1's PSUM evict: the evict is
            # never stuck behind argmin work and PE transposes stay fed.
            kminbig = workp.tile([128, NT * NC], F32, tag="kminbig", bufs=1)
            ps_trs = [None] * NCHUNK

            def argmin_chunk(j):
                ps_tr4 = ps_trs[j]
                ssl = workp.tile([128, TOK], F32, tag="ssl", name=f"ssl{j}")
                nc.vector.scalar_tensor_tensor(
                    out=ssl[:, :].rearrange("p (t f) -> p t f", f=128),
                    in0=ps_tr4[:, :].rearrange("p (t f) -> p t f", f=128),
                    in1=c2_sb[:, 128 * j:128 * (j + 1)].unsqueeze(1)
                        .broadcast_to([128, NT, 128]),
                    scalar=-2.0,
                    op0=mybir.AluOpType.mult,
                    op1=mybir.AluOpType.add,
                )
                ssl4 = ssl[:, :].rearrange("p (t c k) -> p t c k", k=K, c=8)
                m32 = workp.tile([128, NT * 8], F32, tag="m32", name=f"m32_{j}")
                nc.vector.tensor_reduce(
                    out=m32[:, :], in_=ssl4, axis=mybir.AxisListType.X,
                    op=mybir.AluOpType.min,
                )
                eq = workp.tile([128, TOK], F32, tag="eq", name=f"eq{j}")
                nc.vector.tensor_tensor(
                    out=eq[:, :].rearrange("p (t c k) -> p t c k", k=K, c=8),
                    in0=ssl4,
                    in1=m32[:, :].rearrange("p (t c) -> p t c", c=8)
                        .unsqueeze(3).broadcast_to([128, NT, 8, K]),
                    op=mybir.AluOpType.is_equal,
                )
                cand = workp.tile([128, TOK], F32, tag="cand", name=f"cand{j}")
                nc.vector.scalar_tensor_tensor(
                    out=cand[:, :].rearrange("p (t f) -> p t f", f=128),
                    in0=eq[:, :].rearrange("p (t f) -> p t f", f=128),
                    in1=iota_sb[:, :].unsqueeze(1)
                        .broadcast_to([128, NT, 128]),
                    scalar=-BIG,
                    op0=mybir.AluOpType.mult, op1=mybir.AluOpType.add,
                )
                nc.vector.tensor_reduce(
                    out=kminbig[:, :].rearrange(
                        "p (t c) -> p t c", c=NC)[:, :, 8 * j:8 * (j + 1)],
                    in_=cand[:, :].rearrange("p (t c k) -> p t c k", k=K, c=8),
                    axis=mybir.AxisListType.X, op=mybir.AluOpType.min,
                )

            def ship_half(half):
                # kmin [n, c-half] -> bf16 -> transpose -> AllGather.
                # Collective order (AR, AG1, AG2) matches input readiness so
                # the FIFO collective queue never head-of-line blocks.
                c0 = (NC // 2) * half
                kth = constp.tile([NC // 2, TOK], BF16, tag=f"kt_sb{half}",
                                  name=f"kt_sb{half}")
                for t in range(NT):
                    kminb = workp.tile([128, NC // 2], BF16, tag="kminb",
                                       name=f"kminb{half}_{t}")
                    nc.vector.tensor_scalar_add(
                        kminb[:, :].rearrange("p (c j) -> p c j", j=4),
                        kminbig[:, NC * t + c0:NC * t + c0 + NC // 2]
                            .rearrange("p (j c) -> p j c", c=8)
                            .transpose([0, 2, 1]),
                        BIG)
                    ps_kt = psp.tile([NC // 2, 128], BF16, tag="ps",
                                     name=f"ps_kt{half}_{t}")
                    nc.tensor.transpose(ps_kt[:, :], kminb[:, :],
                                        identb[:, :])
                    nc.vector.tensor_copy(
                        out=kth[:, 128 * t:128 * (t + 1)], in_=ps_kt[:, :])
                bnc = kt_bounce1 if half == 0 else kt_bounce2
                gat = kt_all1 if half == 0 else kt_all2
                nc.gpsimd.dma_start(out=bnc[:, :], in_=kth[:, :])
                nc.gpsimd.collective_compute(
                    "AllGather",
                    mybir.AluOpType.bypass,
                    replica_groups=[list(range(N_CORES))],
                    ins=[bnc.opt()],
                    outs=[gat.opt()],
                )

            for j in range(NCHUNK):
                ps_xct = psp.tile([128, TOK], F32, tag="ps", name=f"ps_xct{j}")
                for mcol in range(4):
                    p = 4 * j + mcol
                    xt_t = xtp.tile([128, 2 * TOK], BF16, tag="xt")
                    nc.sync.dma_start(out=xt_t[:, :],
                                      in_=xhl.ap()[128 * p:128 * (p + 1), :])
                    passes = [(cbdh_sb, 0), (cbdh_sb, TOK), (cbdl_sb, 0)]
                    for i, (cb, off) in enumerate(passes):
                        nc.tensor.matmul(
                            ps_xct[32 * mcol:32 * (mcol + 1), :],
                            lhsT=cb[:, 32 * p:32 * (p + 1)],
                            rhs=xt_t[:, off:off + TOK],
                            start=(i == 0), stop=(i == 2),
                            tile_position=(0, 32 * mcol),
                        )
                xct_sb = xctp.tile([128, TOK], F32, tag="xct")
                nc.vector.tensor_copy(out=xct_sb[:, :], in_=ps_xct[:, :])
                ps_tr4 = psp.tile([128, TOK], F32, tag="ps", name=f"ps_tr4_{j}")
                for t in range(NT):
                    nc.tensor.transpose(
                        ps_tr4[:, 128 * t:128 * (t + 1)],
                        xct_sb[:, 128 * t:128 * (t + 1)],
                        identf[:, :],
                    )
                ps_trs[j] = ps_tr4
                if j > 0:
                    argmin_chunk(j - 1)
                    ps_trs[j - 1] = None
                if j == 4:
                    ship_half(0)
            argmin_chunk(NCHUNK - 1)
            ship_half(1)

            # gmax recovery AFTER the AG triggers: the gpsimd stream must not
            # stall on the AllReduce wait before issuing the AllGathers.
            gmax_row = constp.tile([1, 16], F32, tag="gmax_row")
            nc.gpsimd.dma_start(out=gmax_row[:, :], in_=mx_out[:, :])
            gmax = constp.tile([128, 1], F32, tag="gmax")
            nc.gpsimd.partition_broadcast(gmax[:, :], gmax_row[0:1, 0:1])
            # s = gmax/127 and 1/s via reciprocal (DVE has no divide); the
            # <=2ulp drift is far inside the error budget.
            s_col = constp.tile([128, 1], F32, tag="s_col")
            nc.vector.tensor_scalar(
                out=s_col[:, :], in0=gmax[:, :],
                scalar1=float(np.float32(1.0) / np.float32(127.0)),
                scalar2=None, op0=mybir.AluOpType.mult,
            )
            rgmax = constp.tile([128, 1], F32, tag="rgmax")
            nc.vector.reciprocal(rgmax[:, :], gmax[:, :])
            inv_s = constp.tile([128, 1], F32, tag="inv_s")
            nc.vector.tensor_scalar(
                out=inv_s[:, :], in0=rgmax[:, :], scalar1=127.0, scalar2=None,
                op0=mybir.AluOpType.mult,
            )

            # ---- phase 5: quantize-dequantize lut -> bf16 ------------------
            lutdq = lutp.tile([128, NCHUNK * OSH], BF16, tag="lutdq")
            for j in range(NCHUNK):
                qm = wp.tile([128, OSH], F32, tag="qm")
                nc.vector.tensor_scalar(
                    out=qm[:, :], in0=lut_sb[:, OSH * j:OSH * (j + 1)],
                    scalar1=inv_s[:, 0:1], scalar2=MAGIC,
                    op0=mybir.AluOpType.mult, op1=mybir.AluOpType.add,
                )
                nc.vector.tensor_scalar(
                    out=lutdq[:, OSH * j:OSH * (j + 1)], in0=qm[:, :],
                    scalar1=MAGIC, scalar2=s_col[:, 0:1],
                    op0=mybir.AluOpType.subtract, op1=mybir.AluOpType.mult,
                )

            bias_sb = constp.tile([128, OSH], F32, tag="bias_sb")
            nc.sync.dma_start(out=bias_sb[:, :], in_=biasr.ap()[:, :])
            kcol_sb = constp.tile([128, 1], F32, tag="kcol_sb")
            nc.sync.dma_start(out=kcol_sb[:, :], in_=kcol.ap()[:, :])

            # ---- phase 6: expand one-hots + gather matmul ------------------
            # Two half-contractions: chunks 0-3 right after AG1 (overlapping
            # AG2), chunks 4-7 after AG2; bias folded into the first half.
            # One 4D-AP replication DMA + one is_equal per (half, rank).
            half_tiles = {}
            for half in range(2):
                gat = kt_all1 if half == 0 else kt_all2
                for r in range(N_CORES):
                    rep = strepp.tile([128, 4 * TOK], BF16, tag="strep")
                    nc.scalar.dma_start(
                        out=rep[:, :],
                        in_=gat[32 * r:32 * (r + 1), :]
                            .rearrange("(c j) n -> c (j n)", j=4)
                            .unsqueeze(1).broadcast_to([8, K, 4 * TOK]),
                    )
                    stg_t = stgp.tile([128, 4 * TOK], BF16, tag="stg")
                    nc.vector.tensor_scalar(
                        out=stg_t[:, :], in0=rep[:, :],
                        scalar1=kcol_sb[:, 0:1], scalar2=None,
                        op0=mybir.AluOpType.is_equal,
                    )
                    o_sb = None
                    if half == 1:
                        o_sb = outp.tile([128, NT * OSH], BF16, tag="outsb")
                    for t in range(NT):
                        ps_o = psp.tile([128, OSH], F32, tag="ps")
                        for jj in range(NCHUNK // 2):
                            jglob = 4 * half + jj
                            nc.tensor.matmul(
                                ps_o[:, :],
                                lhsT=stg_t[:, TOK * jj + 128 * t:
                                           TOK * jj + 128 * (t + 1)],
                                rhs=lutdq[:, OSH * jglob:OSH * (jglob + 1)],
                                start=(jj == 0), stop=(jj == NCHUNK // 2 - 1),
                            )
                        if half == 0:
                            hs = halfp.tile([128, OSH], BF16, tag="hsum",
                                            name=f"hs{r}_{t}")
                            nc.vector.tensor_tensor(
                                out=hs[:, :], in0=ps_o[:, :], in1=bias_sb[:, :],
                                op=mybir.AluOpType.add,
                            )
                            half_tiles[(r, t)] = hs
                        else:
                            nc.vector.tensor_tensor(
                                out=o_sb[:, OSH * t:OSH * (t + 1)],
                                in0=ps_o[:, :], in1=half_tiles[(r, t)][:, :],
                                op=mybir.AluOpType.add,
                            )
                    if half == 1:
                        nc.sync.dma_start(
                            out=out.ap()[TOK * r:TOK * (r + 1), :]
                                .rearrange("(t p) o -> p t o", p=128),
                            in_=o_sb[:, :].rearrange("p (t o) -> p t o", o=OSH),
                        )

    nc.compile()
    return nc


def _prep_inputs(x, centroids, weight, bias):
    import ml_dtypes

    x = np.ascontiguousarray(np.asarray(x, dtype=np.float32)).reshape(BN, IN_F)
    cent = np.asarray(centroids, dtype=np.float32)
    w = np.asarray(weight, dtype=np.float32)
    bias = np.asarray(bias, dtype=np.float32)

    c2 = (cent ** 2).sum(axis=-1).reshape(CK)  # [1024] flat (c,k)
    c2r = np.ascontiguousarray(np.broadcast_to(c2, (128, CK)))
    iota = np.tile(np.arange(K, dtype=np.float32), 8)
    iotar = np.ascontiguousarray(np.broadcast_to(iota, (128, 128)))
    cbd = np.zeros((128, CK), np.float32)
    for p in range(NPAIR):
        cbd[0:SUBV, 32 * p:32 * p + K] = cent[2 * p].T
        cbd[SUBV:128, 32 * p + K:32 * p + 2 * K] = cent[2 * p + 1].T
    cbd_h = cbd.astype(ml_dtypes.bfloat16)
    cbd_l = (cbd - cbd_h.astype(np.float32)).astype(ml_dtypes.bfloat16)
    cbd_f16 = cbd.astype(np.float16)
    kcol = np.ascontiguousarray(
        (np.arange(128, dtype=np.float32) % K).reshape(128, 1))

    in_maps = []
    for r in range(N_CORES):
        xT_r = np.ascontiguousarray(x[TOK * r:TOK * (r + 1)].T)
        xh_r = xT_r.astype(ml_dtypes.bfloat16)
        xl_r = (xT_r - xh_r.astype(np.float32)).astype(ml_dtypes.bfloat16)
        xhl_r = np.ascontiguousarray(np.concatenate([xh_r, xl_r], axis=1))
        w_r = np.ascontiguousarray(w[:, :, OSH * r:OSH * (r + 1)]).reshape(
            IN_F, OSH)
        wf16_r = w_r.astype(np.float16)
        bias_r = np.ascontiguousarray(
            np.broadcast_to(bias[OSH * r:OSH * (r + 1)], (128, OSH)))
        in_maps.append({
            "xhl": xhl_r, "wf16": wf16_r, "cbdf16": cbd_f16,
            "cbd_h": cbd_h, "cbd_l": cbd_l,
            "c2r": c2r, "iotar": iotar, "biasr": bias_r, "kcol": kcol,
        })
    return in_maps


def kernel(x, centroids, weight, inverse_temperature_logit, bias,
           **_unused) -> np.ndarray:
    if "nc" not in _CACHE:
        _CACHE["nc"] = _build()
    nc = _CACHE["nc"]
    in_maps = _prep_inputs(x, centroids, weight, bias)
    res = run_bass_kernel_spmd(nc, in_maps, core_ids=list(range(N_CORES)))
    out = np.concatenate(
        [res.results[r]["out"].astype(np.float32) for r in range(N_CORES)],
        axis=1)
    return out.reshape(2, BN // 2, OUT_F)


# revision 38
# speedup vs baseline: 1.1542x; 1.0303x over previous
"""AMMLinear (VQ codebook) forward on 8 TRN2 NeuronCores.

The straight-through estimator makes the forward VALUE exactly
    out[n, o] = sum_c lut_dq[c, argmin_k dist(x_cn, cent_ck), o] + bias[o]
with lut = centroids @ weight (per codebook) and lut_dq a global-scale int8
quantize-dequantize of lut.  The softmax/attention path only shapes gradients.

Sharding: tokens (BN=4096 -> 512/core) for the score/argmin phase, output
features (4096 -> 512/core) for the lut/gather phase.  One AllGather moves the
bf16 argmin indices (64KB/core), one AllReduce-max the quantization scale.
Every core then expands all 4096 tokens' one-hot codes locally (replication
DMA + is_equal) and computes its o-shard of the gather matmul.
Output is assembled host-side by concatenating the per-core o-shards.

Numerics: scores use 3-pass bf16 hi/lo matmuls (a*b ~ ah*bh + ah*bl + al*bh,
fp32 PSUM accumulate, ~2^-18/product) to keep the argmin faithful; the lut
uses a single fp16 pass (~2^-11/product, inside the int8-quantization error
budget); the gather matmul runs in bf16 (exact one-hots, bf16-rounded
lut_dq); output in bf16.  Measured rel err ~6e-3 vs the fp32 reference.
"""

import numpy as np

import concourse.bass as bass
import concourse.mybir as mybir
import concourse.tile as tile
import concourse.bass_isa as bass_isa
from concourse import bacc
from concourse.bass_utils import run_bass_kernel_spmd
from concourse.masks import make_identity

F32 = mybir.dt.float32
BF16 = mybir.dt.bfloat16

N_CORES = 8
NC, K, IN_F, OUT_F = 64, 16, 4096, 4096
SUBV = IN_F // NC          # 64
BN = 4096                  # 2*2048 tokens
TOK = BN // N_CORES        # 512 tokens per core
NT = TOK // 128            # 4 token tiles per core
NPAIR = NC // 2            # 32 codebook pairs
CK = NC * K                # 1024 (codebook,centroid) flat index
NCHUNK = CK // 128         # 8 contraction chunks
OSH = OUT_F // N_CORES     # 512 out features per core
MAGIC = 12582912.0         # 1.5 * 2**23: fp32 round-to-nearest-even trick
BIG = 1024.0

_CACHE = {}


def _build():
    nc = bacc.Bacc("TRN2", target_bir_lowering=False, debug=False,
                   num_devices=N_CORES)

    # xhl[:, 0:TOK] = bf16 hi of x^T shard, [:, TOK:2*TOK] = bf16 lo
    xhl = nc.declare_dram_parameter("xhl", [IN_F, 2 * TOK], BF16,
                                    isOutput=False)
    # w o-shard in fp16 (single-pass lut: ~2^-11/product, inside budget)
    wf16 = nc.declare_dram_parameter("wf16", [IN_F, OSH], mybir.dt.float16,
                                     isOutput=False)
    cbdf16 = nc.declare_dram_parameter("cbdf16", [128, CK], mybir.dt.float16,
                                       isOutput=False)
    cbd_h = nc.declare_dram_parameter("cbd_h", [128, CK], BF16, isOutput=False)
    cbd_l = nc.declare_dram_parameter("cbd_l", [128, CK], BF16, isOutput=False)
    c2r = nc.declare_dram_parameter("c2r", [128, CK], F32, isOutput=False)
    iotar = nc.declare_dram_parameter("iotar", [128, 128], F32, isOutput=False)
    biasr = nc.declare_dram_parameter("biasr", [128, OSH], F32, isOutput=False)
    kcol = nc.declare_dram_parameter("kcol", [128, 1], F32, isOutput=False)
    out = nc.declare_dram_parameter("out", [BN, OSH], BF16, isOutput=True)

    with tile.TileContext(nc) as tc:
        with (
            tc.tile_pool(name="consts", bufs=1) as constp,
            tc.tile_pool(name="xt", bufs=6) as xtp,
            tc.tile_pool(name="wt", bufs=16) as wp,
            tc.tile_pool(name="xct", bufs=3) as xctp,
            tc.tile_pool(name="work", bufs=2) as workp,
            tc.tile_pool(name="stg", bufs=8) as stgp,
            tc.tile_pool(name="strep", bufs=6) as strepp,
            tc.tile_pool(name="half", bufs=32) as halfp,
            tc.tile_pool(name="lut", bufs=1) as lutp,
            tc.tile_pool(name="outs", bufs=2) as outp,
            tc.tile_pool(name="ps", bufs=8, space="PSUM") as psp,
            tc.tile_pool(name="dram", bufs=1, space="DRAM") as dramp,
        ):
            # ---- constants -------------------------------------------------
            cbdf_sb = constp.tile([128, CK], mybir.dt.float16, tag="cbdf_sb")
            nc.sync.dma_start(out=cbdf_sb[:, :], in_=cbdf16.ap()[:, :])
            cbdh_sb = constp.tile([128, CK], BF16, tag="cbdh_sb")
            nc.sync.dma_start(out=cbdh_sb[:, :], in_=cbd_h.ap()[:, :])
            cbdl_sb = constp.tile([128, CK], BF16, tag="cbdl_sb")
            nc.sync.dma_start(out=cbdl_sb[:, :], in_=cbd_l.ap()[:, :])
            c2_sb = constp.tile([128, CK], F32, tag="c2_sb")
            nc.sync.dma_start(out=c2_sb[:, :], in_=c2r.ap()[:, :])
            iota_sb = constp.tile([128, 128], F32, tag="iota_sb")
            nc.sync.dma_start(out=iota_sb[:, :], in_=iotar.ap()[:, :])
            identb = constp.tile([128, 128], BF16, tag="identb")
            make_identity(nc, identb[:, :])
            identf = constp.tile([128, 128], F32, tag="identf")
            make_identity(nc, identf[:, :])

            # DRAM scratch for collectives
            kt_bounce1 = dramp.tile([NC // 2, TOK], BF16, tag="kt_bounce1")
            kt_bounce2 = dramp.tile([NC // 2, TOK], BF16, tag="kt_bounce2")
            kt_all1 = dramp.tile([N_CORES * NC // 2, TOK], BF16, tag="kt_all1")
            kt_all2 = dramp.tile([N_CORES * NC // 2, TOK], BF16, tag="kt_all2")
            mx_in = dramp.tile([1, 16], F32, tag="mx_in")
            mx_out = dramp.tile([1, 16], F32, tag="mx_out")

            # ---- phase A: lut = centroids @ weight (3-pass bf16 hi/lo) ----
            # Runs first so the absmax -> AllReduce(max) chain is issued as
            # early as possible (the collective queue is FIFO: the first
            # collective blocks the second until it completes).
            lut_sb = lutp.tile([128, NCHUNK * OSH], F32, tag="lut")
            mx8 = constp.tile([128, NCHUNK], F32, tag="mx8")
            for j in range(NCHUNK):
                ps_lut = psp.tile([128, OSH], F32, tag="ps")
                for mcol in range(4):
                    p = 4 * j + mcol
                    w_t = wp.tile([128, OSH], mybir.dt.float16, tag="wt")
                    nc.scalar.dma_start(
                        out=w_t[:, :],
                        in_=wf16.ap()[128 * p:128 * (p + 1), :])
                    nc.tensor.matmul(
                        ps_lut[32 * mcol:32 * (mcol + 1), :],
                        lhsT=cbdf_sb[:, 32 * p:32 * (p + 1)],
                        rhs=w_t[:, :],
                        start=True, stop=True,
                        tile_position=(0, 32 * mcol),
                    )
                nc.vector.tensor_copy(
                    out=lut_sb[:, OSH * j:OSH * (j + 1)], in_=ps_lut[:, :])
                nc.vector.tensor_reduce(
                    out=mx8[:, j:j + 1],
                    in_=lut_sb[:, OSH * j:OSH * (j + 1)],
                    axis=mybir.AxisListType.X, op=mybir.AluOpType.max,
                    apply_absolute_value=True,
                )

            # ---- phase B: global absmax -> AllReduce(max) -> scale ---------
            mxl = constp.tile([128, 1], F32, tag="mxl")
            nc.vector.tensor_reduce(
                out=mxl[:, :], in_=mx8[:, :], axis=mybir.AxisListType.X,
                op=mybir.AluOpType.max,
            )
            mxp = constp.tile([128, 1], F32, tag="mxp")
            nc.gpsimd.partition_all_reduce(
                mxp[:, :], mxl[:, :], channels=128,
                reduce_op=bass_isa.ReduceOp.max,
            )
            zrow = constp.tile([1, 16], F32, tag="zrow")
            nc.vector.memset(zrow[:, :], 0.0)
            nc.vector.tensor_copy(out=zrow[0:1, 0:1], in_=mxp[0:1, 0:1])
            nc.gpsimd.dma_start(out=mx_in[:, :], in_=zrow[:, :])
            nc.gpsimd.collective_compute(
                "AllReduce",
                mybir.AluOpType.max,
                replica_groups=[list(range(N_CORES))],
                ins=[mx_in.opt()],
                outs=[mx_out.opt()],
            )

            # ---- phase C: scores + incremental argmin ----------------------
            # The DVE executes its stream in order, so the argmin chain for
            # chunk j is emitted AFTER chunk j+1's PSUM evict: the evict is
            # never stuck behind argmin work and PE transposes stay fed.
            kminbig = workp.tile([128, NT * NC], F32, tag="kminbig", bufs=1)
            ps_trs = [None] * NCHUNK

            def argmin_chunk(j):
                ps_tr4 = ps_trs[j]
                ssl = workp.tile([128, TOK], F32, tag="ssl", name=f"ssl{j}")
                nc.vector.scalar_tensor_tensor(
                    out=ssl[:, :].rearrange("p (t f) -> p t f", f=128),
                    in0=ps_tr4[:, :].rearrange("p (t f) -> p t f", f=128),
                    in1=c2_sb[:, 128 * j:128 * (j + 1)].unsqueeze(1)
                        .broadcast_to([128, NT, 128]),
                    scalar=-2.0,
                    op0=mybir.AluOpType.mult,
                    op1=mybir.AluOpType.add,
                )
                ssl4 = ssl[:, :].rearrange("p (t c k) -> p t c k", k=K, c=8)
                m32 = workp.tile([128, NT * 8], F32, tag="m32", name=f"m32_{j}")
                nc.vector.tensor_reduce(
                    out=m32[:, :], in_=ssl4, axis=mybir.AxisListType.X,
                    op=mybir.AluOpType.min,
                )
                eq = workp.tile([128, TOK], F32, tag="eq", name=f"eq{j}")
                nc.vector.tensor_tensor(
                    out=eq[:, :].rearrange("p (t c k) -> p t c k", k=K, c=8),
                    in0=ssl4,
                    in1=m32[:, :].rearrange("p (t c) -> p t c", c=8)
                        .unsqueeze(3).broadcast_to([128, NT, 8, K]),
                    op=mybir.AluOpType.is_equal,
                )
                cand = workp.tile([128, TOK], F32, tag="cand", name=f"cand{j}")
                nc.vector.scalar_tensor_tensor(
                    out=cand[:, :].rearrange("p (t f) -> p t f", f=128),
                    in0=eq[:, :].rearrange("p (t f) -> p t f", f=128),
                    in1=iota_sb[:, :].unsqueeze(1)
                        .broadcast_to([128, NT, 128]),
                    scalar=-BIG,
                    op0=mybir.AluOpType.mult, op1=mybir.AluOpType.add,
                )
                nc.vector.tensor_reduce(
                    out=kminbig[:, :].rearrange(
                        "p (t c) -> p t c", c=NC)[:, :, 8 * j:8 * (j + 1)],
                    in_=cand[:, :].rearrange("p (t c k) -> p t c k", k=K, c=8),
                    axis=mybir.AxisListType.X, op=mybir.AluOpType.min,
                )

            def ship_half(half):
                # kmin [n, c-half] -> bf16 -> transpose -> AllGather.
                # Collective order (AR, AG1, AG2) matches input readiness so
                # the FIFO collective queue never head-of-line blocks.
                c0 = (NC // 2) * half
                kth = constp.tile([NC // 2, TOK], BF16, tag=f"kt_sb{half}",
                                  name=f"kt_sb{half}")
                for t in range(NT):
                    kminb = workp.tile([128, NC // 2], BF16, tag="kminb",
                                       name=f"kminb{half}_{t}")
                    nc.vector.tensor_scalar_add(
                        kminb[:, :].rearrange("p (c j) -> p c j", j=4),
                        kminbig[:, NC * t + c0:NC * t + c0 + NC // 2]
                            .rearrange("p (j c) -> p j c", c=8)
                            .transpose([0, 2, 1]),
                        BIG)
                    ps_kt = psp.tile([NC // 2, 128], BF16, tag="ps",
                                     name=f"ps_kt{half}_{t}")
                    nc.tensor.transpose(ps_kt[:, :], kminb[:, :],
                                        identb[:, :])
                    nc.vector.tensor_copy(
                        out=kth[:, 128 * t:128 * (t + 1)], in_=ps_kt[:, :])
                bnc = kt_bounce1 if half == 0 else kt_bounce2
                gat = kt_all1 if half == 0 else kt_all2
                nc.gpsimd.dma_start(out=bnc[:, :], in_=kth[:, :])
                nc.gpsimd.collective_compute(
                    "AllGather",
                    mybir.AluOpType.bypass,
                    replica_groups=[list(range(N_CORES))],
                    ins=[bnc.opt()],
                    outs=[gat.opt()],
                )

            for j in range(NCHUNK):
                ps_xct = psp.tile([128, TOK], F32, tag="ps", name=f"ps_xct{j}")
                for mcol in range(4):
                    p = 4 * j + mcol
                    xt_t = xtp.tile([128, 2 * TOK], BF16, tag="xt")
                    nc.sync.dma_start(out=xt_t[:, :],
                                      in_=xhl.ap()[128 * p:128 * (p + 1), :])
                    passes = [(cbdh_sb, 0), (cbdh_sb, TOK), (cbdl_sb, 0)]
                    for i, (cb, off) in enumerate(passes):
                        nc.tensor.matmul(
                            ps_xct[32 * mcol:32 * (mcol + 1), :],
                            lhsT=cb[:, 32 * p:32 * (p + 1)],
                            rhs=xt_t[:, off:off + TOK],
                            start=(i == 0), stop=(i == 2),
                            tile_position=(0, 32 * mcol),
                        )
                xct_sb = xctp.tile([128, TOK], F32, tag="xct")
                nc.vector.tensor_copy(out=xct_sb[:, :], in_=ps_xct[:, :])
                ps_tr4 = psp.tile([128, TOK], F32, tag="ps", name=f"ps_tr4_{j}")
                for t in range(NT):
                    nc.tensor.transpose(
                        ps_tr4[:, 128 * t:128 * (t + 1)],
                        xct_sb[:, 128 * t:128 * (t + 1)],
                        identf[:, :],
                    )
                ps_trs[j] = ps_tr4
                if j > 0:
                    argmin_chunk(j - 1)
                    ps_trs[j - 1] = None
                if j == 4:
                    ship_half(0)
            argmin_chunk(NCHUNK - 1)
            ship_half(1)

            # gmax recovery AFTER the AG triggers: the gpsimd stream must not
            # stall on the AllReduce wait before issuing the AllGathers.
            gmax_row = constp.tile([1, 16], F32, tag="gmax_row")
            nc.gpsimd.dma_start(out=gmax_row[:, :], in_=mx_out[:, :])
            gmax = constp.tile([128, 1], F32, tag="gmax")
            nc.gpsimd.partition_broadcast(gmax[:, :], gmax_row[0:1, 0:1])
            # s = gmax/127 and 1/s via reciprocal (DVE has no divide); the
            # <=2ulp drift is far inside the error budget.
            s_col = constp.tile([128, 1], F32, tag="s_col")
            nc.vector.tensor_scalar(
                out=s_col[:, :], in0=gmax[:, :],
                scalar1=float(np.float32(1.0) / np.float32(127.0)),
                scalar2=None, op0=mybir.AluOpType.mult,
            )
            rgmax = constp.tile([128, 1], F32, tag="rgmax")
            nc.vector.reciprocal(rgmax[:, :], gmax[:, :])
            inv_s = constp.tile([128, 1], F32, tag="inv_s")
            nc.vector.tensor_scalar(
                out=inv_s[:, :], in0=rgmax[:, :], scalar1=127.0, scalar2=None,
                op0=mybir.AluOpType.mult,
            )

            # ---- phase 5: quantize-dequantize lut -> bf16 ------------------
            lutdq = lutp.tile([128, NCHUNK * OSH], BF16, tag="lutdq")
            for j in range(NCHUNK):
                qm = workp.tile([128, OSH], F32, tag="qm")
                nc.vector.tensor_scalar(
                    out=qm[:, :], in0=lut_sb[:, OSH * j:OSH * (j + 1)],
                    scalar1=inv_s[:, 0:1], scalar2=MAGIC,
                    op0=mybir.AluOpType.mult, op1=mybir.AluOpType.add,
                )
                nc.vector.tensor_scalar(
                    out=lutdq[:, OSH * j:OSH * (j + 1)], in0=qm[:, :],
                    scalar1=MAGIC, scalar2=s_col[:, 0:1],
                    op0=mybir.AluOpType.subtract, op1=mybir.AluOpType.mult,
                )

            bias_sb = constp.tile([128, OSH], F32, tag="bias_sb")
            nc.sync.dma_start(out=bias_sb[:, :], in_=biasr.ap()[:, :])
            kcol_sb = constp.tile([128, 1], F32, tag="kcol_sb")
            nc.sync.dma_start(out=kcol_sb[:, :], in_=kcol.ap()[:, :])

            # ---- phase 6: expand one-hots + gather matmul ------------------
            # Two half-contractions: chunks 0-3 right after AG1 (overlapping
            # AG2), chunks 4-7 after AG2; bias folded into the first half.
            # One 4D-AP replication DMA + one is_equal per (half, rank).
            half_tiles = {}
            for half in range(2):
                gat = kt_all1 if half == 0 else kt_all2
                for r in range(N_CORES):
                    rep = strepp.tile([128, 4 * TOK], BF16, tag="strep")
                    nc.scalar.dma_start(
                        out=rep[:, :],
                        in_=gat[32 * r:32 * (r + 1), :]
                            .rearrange("(c j) n -> c (j n)", j=4)
                            .unsqueeze(1).broadcast_to([8, K, 4 * TOK]),
                    )
                    stg_t = stgp.tile([128, 4 * TOK], BF16, tag="stg")
                    nc.vector.tensor_scalar(
                        out=stg_t[:, :], in0=rep[:, :],
                        scalar1=kcol_sb[:, 0:1], scalar2=None,
                        op0=mybir.AluOpType.is_equal,
                    )
                    o_sb = None
                    if half == 1:
                        o_sb = outp.tile([128, NT * OSH], BF16, tag="outsb")
                    for t in range(NT):
                        ps_o = psp.tile([128, OSH], F32, tag="ps")
                        for jj in range(NCHUNK // 2):
                            jglob = 4 * half + jj
                            nc.tensor.matmul(
                                ps_o[:, :],
                                lhsT=stg_t[:, TOK * jj + 128 * t:
                                           TOK * jj + 128 * (t + 1)],
                                rhs=lutdq[:, OSH * jglob:OSH * (jglob + 1)],
                                start=(jj == 0), stop=(jj == NCHUNK // 2 - 1),
                            )
                        if half == 0:
                            hs = halfp.tile([128, OSH], BF16, tag="hsum",
                                            name=f"hs{r}_{t}")
                            nc.vector.tensor_tensor(
                                out=hs[:, :], in0=ps_o[:, :], in1=bias_sb[:, :],
                                op=mybir.AluOpType.add,
                            )
                            half_tiles[(r, t)] = hs
                        else:
                            nc.vector.tensor_tensor(
                                out=o_sb[:, OSH * t:OSH * (t + 1)],
                                in0=ps_o[:, :], in1=half_tiles[(r, t)][:, :],
                                op=mybir.AluOpType.add,
                            )
                    if half == 1:
                        nc.sync.dma_start(
                            out=out.ap()[TOK * r:TOK * (r + 1), :]
                                .rearrange("(t p) o -> p t o", p=128),
                            in_=o_sb[:, :].rearrange("p (t o) -> p t o", o=OSH),
                        )

    nc.compile()
    return nc


def _prep_inputs(x, centroids, weight, bias):
    import ml_dtypes

    x = np.ascontiguousarray(np.asarray(x, dtype=np.float32)).reshape(BN, IN_F)
    cent = np.asarray(centroids, dtype=np.float32)
    w = np.asarray(weight, dtype=np.float32)
    bias = np.asarray(bias, dtype=np.float32)

    c2 = (cent ** 2).sum(axis=-1).reshape(CK)  # [1024] flat (c,k)
    c2r = np.ascontiguousarray(np.broadcast_to(c2, (128, CK)))
    iota = np.tile(np.arange(K, dtype=np.float32), 8)
    iotar = np.ascontiguousarray(np.broadcast_to(iota, (128, 128)))
    cbd = np.zeros((128, CK), np.float32)
    for p in range(NPAIR):
        cbd[0:SUBV, 32 * p:32 * p + K] = cent[2 * p].T
        cbd[SUBV:128, 32 * p + K:32 * p + 2 * K] = cent[2 * p + 1].T
    cbd_h = cbd.astype(ml_dtypes.bfloat16)
    cbd_l = (cbd - cbd_h.astype(np.float32)).astype(ml_dtypes.bfloat16)
    cbd_f16 = cbd.astype(np.float16)
    kcol = np.ascontiguousarray(
        (np.arange(128, dtype=np.float32) % K).reshape(128, 1))

    in_maps = []
    for r in range(N_CORES):
        xT_r = np.ascontiguousarray(x[TOK * r:TOK * (r + 1)].T)
        xh_r = xT_r.astype(ml_dtypes.bfloat16)
        xl_r = (xT_r - xh_r.astype(np.float32)).astype(ml_dtypes.bfloat16)
        xhl_r = np.ascontiguousarray(np.concatenate([xh_r, xl_r], axis=1))
        w_r = np.ascontiguousarray(w[:, :, OSH * r:OSH * (r + 1)]).reshape(
            IN_F, OSH)
        wf16_r = w_r.astype(np.float16)
        bias_r = np.ascontiguousarray(
            np.broadcast_to(bias[OSH * r:OSH * (r + 1)], (128, OSH)))
        in_maps.append({
            "xhl": xhl_r, "wf16": wf16_r, "cbdf16": cbd_f16,
            "cbd_h": cbd_h, "cbd_l": cbd_l,
            "c2r": c2r, "iotar": iotar, "biasr": bias_r, "kcol": kcol,
        })
    return in_maps


def kernel(x, centroids, weight, inverse_temperature_logit, bias,
           **_unused) -> np.ndarray:
    if "nc" not in _CACHE:
        _CACHE["nc"] = _build()
    nc = _CACHE["nc"]
    in_maps = _prep_inputs(x, centroids, weight, bias)
    res = run_bass_kernel_spmd(nc, in_maps, core_ids=list(range(N_CORES)))
    out = np.concatenate(
        [res.results[r]["out"].astype(np.float32) for r in range(N_CORES)],
        axis=1)
    return out.reshape(2, BN // 2, OUT_F)


# revision 39
# speedup vs baseline: 1.2224x; 1.0591x over previous
"""AMMLinear (VQ codebook) forward on 8 TRN2 NeuronCores.

The straight-through estimator makes the forward VALUE exactly
    out[n, o] = sum_c lut_dq[c, argmin_k dist(x_cn, cent_ck), o] + bias[o]
with lut = centroids @ weight (per codebook) and lut_dq a global-scale int8
quantize-dequantize of lut.  The softmax/attention path only shapes gradients.

Sharding: tokens (BN=4096 -> 512/core) for the score/argmin phase, output
features (4096 -> 512/core) for the lut/gather phase.  One AllGather moves the
bf16 argmin indices (64KB/core), one AllReduce-max the quantization scale.
Every core then expands all 4096 tokens' one-hot codes locally (replication
DMA + is_equal) and computes its o-shard of the gather matmul.
Output is assembled host-side by concatenating the per-core o-shards.

Numerics: scores use 3-pass bf16 hi/lo matmuls (a*b ~ ah*bh + ah*bl + al*bh,
fp32 PSUM accumulate, ~2^-18/product) to keep the argmin faithful; the lut
uses a single fp16 pass (~2^-11/product, inside the int8-quantization error
budget); the gather matmul runs in bf16 (exact one-hots, bf16-rounded
lut_dq); output in bf16.  Measured rel err ~6e-3 vs the fp32 reference.
"""

import numpy as np

import concourse.bass as bass
import concourse.mybir as mybir
import concourse.tile as tile
import concourse.bass_isa as bass_isa
from concourse import bacc
from concourse.bass_utils import run_bass_kernel_spmd
from concourse.masks import make_identity

F32 = mybir.dt.float32
BF16 = mybir.dt.bfloat16

N_CORES = 8
NC, K, IN_F, OUT_F = 64, 16, 4096, 4096
SUBV = IN_F // NC          # 64
BN = 4096                  # 2*2048 tokens
TOK = BN // N_CORES        # 512 tokens per core
NT = TOK // 128            # 4 token tiles per core
NPAIR = NC // 2            # 32 codebook pairs
CK = NC * K                # 1024 (codebook,centroid) flat index
NCHUNK = CK // 128         # 8 contraction chunks
OSH = OUT_F // N_CORES     # 512 out features per core
MAGIC = 12582912.0         # 1.5 * 2**23: fp32 round-to-nearest-even trick
BIG = 1024.0

_CACHE = {}


def _build():
    nc = bacc.Bacc("TRN2", target_bir_lowering=False, debug=False,
                   num_devices=N_CORES)

    # xhl[:, 0:TOK] = bf16 hi of x^T shard, [:, TOK:2*TOK] = bf16 lo
    xhl = nc.declare_dram_parameter("xhl", [IN_F, 2 * TOK], BF16,
                                    isOutput=False)
    # w o-shard in fp16 (single-pass lut: ~2^-11/product, inside budget)
    wf16 = nc.declare_dram_parameter("wf16", [IN_F, OSH], mybir.dt.float16,
                                     isOutput=False)
    cbdf16 = nc.declare_dram_parameter("cbdf16", [128, CK], mybir.dt.float16,
                                       isOutput=False)
    cbd_h = nc.declare_dram_parameter("cbd_h", [128, CK], BF16, isOutput=False)
    cbd_l = nc.declare_dram_parameter("cbd_l", [128, CK], BF16, isOutput=False)
    c2r = nc.declare_dram_parameter("c2r", [128, CK], F32, isOutput=False)
    iotar = nc.declare_dram_parameter("iotar", [128, 128], F32, isOutput=False)
    biasr = nc.declare_dram_parameter("biasr", [128, OSH], F32, isOutput=False)
    kcol = nc.declare_dram_parameter("kcol", [128, 1], F32, isOutput=False)
    out = nc.declare_dram_parameter("out", [BN, OSH], BF16, isOutput=True)

    with tile.TileContext(nc) as tc:
        with (
            tc.tile_pool(name="consts", bufs=1) as constp,
            tc.tile_pool(name="xt", bufs=10) as xtp,
            tc.tile_pool(name="wt", bufs=16) as wp,
            tc.tile_pool(name="xct", bufs=3) as xctp,
            tc.tile_pool(name="work", bufs=2) as workp,
            tc.tile_pool(name="stg", bufs=8) as stgp,
            tc.tile_pool(name="strep", bufs=6) as strepp,
            tc.tile_pool(name="half", bufs=32) as halfp,
            tc.tile_pool(name="lut", bufs=1) as lutp,
            tc.tile_pool(name="outs", bufs=2) as outp,
            tc.tile_pool(name="ps", bufs=8, space="PSUM") as psp,
            tc.tile_pool(name="dram", bufs=1, space="DRAM") as dramp,
        ):
            # ---- constants -------------------------------------------------
            cbdf_sb = constp.tile([128, CK], mybir.dt.float16, tag="cbdf_sb")
            nc.sync.dma_start(out=cbdf_sb[:, :], in_=cbdf16.ap()[:, :])
            cbdh_sb = constp.tile([128, CK], BF16, tag="cbdh_sb")
            nc.sync.dma_start(out=cbdh_sb[:, :], in_=cbd_h.ap()[:, :])
            cbdl_sb = constp.tile([128, CK], BF16, tag="cbdl_sb")
            nc.sync.dma_start(out=cbdl_sb[:, :], in_=cbd_l.ap()[:, :])
            c2_sb = constp.tile([128, CK], F32, tag="c2_sb")
            nc.sync.dma_start(out=c2_sb[:, :], in_=c2r.ap()[:, :])
            iota_sb = constp.tile([128, 128], F32, tag="iota_sb")
            nc.sync.dma_start(out=iota_sb[:, :], in_=iotar.ap()[:, :])
            identb = constp.tile([128, 128], BF16, tag="identb")
            make_identity(nc, identb[:, :])
            identf = constp.tile([128, 128], F32, tag="identf")
            make_identity(nc, identf[:, :])

            # DRAM scratch for collectives
            kt_bounce1 = dramp.tile([NC // 2, TOK], BF16, tag="kt_bounce1")
            kt_bounce2 = dramp.tile([NC // 2, TOK], BF16, tag="kt_bounce2")
            kt_all1 = dramp.tile([N_CORES * NC // 2, TOK], BF16, tag="kt_all1")
            kt_all2 = dramp.tile([N_CORES * NC // 2, TOK], BF16, tag="kt_all2")
            mx_in = dramp.tile([1, 16], F32, tag="mx_in")
            mx_out = dramp.tile([1, 16], F32, tag="mx_out")

            # ---- phase A: lut = centroids @ weight (3-pass bf16 hi/lo) ----
            # Runs first so the absmax -> AllReduce(max) chain is issued as
            # early as possible (the collective queue is FIFO: the first
            # collective blocks the second until it completes).
            lut_sb = lutp.tile([128, NCHUNK * OSH], F32, tag="lut")
            mx8 = constp.tile([128, NCHUNK], F32, tag="mx8")
            for j in range(NCHUNK):
                ps_lut = psp.tile([128, OSH], F32, tag="ps")
                for mcol in range(4):
                    p = 4 * j + mcol
                    w_t = wp.tile([128, OSH], mybir.dt.float16, tag="wt")
                    nc.scalar.dma_start(
                        out=w_t[:, :],
                        in_=wf16.ap()[128 * p:128 * (p + 1), :])
                    nc.tensor.matmul(
                        ps_lut[32 * mcol:32 * (mcol + 1), :],
                        lhsT=cbdf_sb[:, 32 * p:32 * (p + 1)],
                        rhs=w_t[:, :],
                        start=True, stop=True,
                        tile_position=(0, 32 * mcol),
                    )
                nc.vector.tensor_copy(
                    out=lut_sb[:, OSH * j:OSH * (j + 1)], in_=ps_lut[:, :])
                nc.vector.tensor_reduce(
                    out=mx8[:, j:j + 1],
                    in_=lut_sb[:, OSH * j:OSH * (j + 1)],
                    axis=mybir.AxisListType.X, op=mybir.AluOpType.max,
                    apply_absolute_value=True,
                )

            # ---- phase B: global absmax -> AllReduce(max) -> scale ---------
            mxl = constp.tile([128, 1], F32, tag="mxl")
            nc.vector.tensor_reduce(
                out=mxl[:, :], in_=mx8[:, :], axis=mybir.AxisListType.X,
                op=mybir.AluOpType.max,
            )
            mxp = constp.tile([128, 1], F32, tag="mxp")
            nc.gpsimd.partition_all_reduce(
                mxp[:, :], mxl[:, :], channels=128,
                reduce_op=bass_isa.ReduceOp.max,
            )
            zrow = constp.tile([1, 16], F32, tag="zrow")
            nc.vector.memset(zrow[:, :], 0.0)
            nc.vector.tensor_copy(out=zrow[0:1, 0:1], in_=mxp[0:1, 0:1])
            nc.gpsimd.dma_start(out=mx_in[:, :], in_=zrow[:, :])
            nc.gpsimd.collective_compute(
                "AllReduce",
                mybir.AluOpType.max,
                replica_groups=[list(range(N_CORES))],
                ins=[mx_in.opt()],
                outs=[mx_out.opt()],
            )

            # ---- phase C: scores + incremental argmin ----------------------
            # The DVE executes its stream in order, so the argmin chain for
            # chunk j is emitted AFTER chunk j+1's PSUM evict: the evict is
            # never stuck behind argmin work and PE transposes stay fed.
            kminbig = workp.tile([128, NT * NC], F32, tag="kminbig", bufs=1)
            ps_trs = [None] * NCHUNK

            def argmin_chunk(j):
                ps_tr4 = ps_trs[j]
                ssl = workp.tile([128, TOK], F32, tag="ssl", name=f"ssl{j}")
                nc.vector.scalar_tensor_tensor(
                    out=ssl[:, :].rearrange("p (t f) -> p t f", f=128),
                    in0=ps_tr4[:, :].rearrange("p (t f) -> p t f", f=128),
                    in1=c2_sb[:, 128 * j:128 * (j + 1)].unsqueeze(1)
                        .broadcast_to([128, NT, 128]),
                    scalar=-2.0,
                    op0=mybir.AluOpType.mult,
                    op1=mybir.AluOpType.add,
                )
                ssl4 = ssl[:, :].rearrange("p (t c k) -> p t c k", k=K, c=8)
                m32 = workp.tile([128, NT * 8], F32, tag="m32", name=f"m32_{j}")
                nc.vector.tensor_reduce(
                    out=m32[:, :], in_=ssl4, axis=mybir.AxisListType.X,
                    op=mybir.AluOpType.min,
                )
                eq = workp.tile([128, TOK], F32, tag="eq", name=f"eq{j}")
                nc.vector.tensor_tensor(
                    out=eq[:, :].rearrange("p (t c k) -> p t c k", k=K, c=8),
                    in0=ssl4,
                    in1=m32[:, :].rearrange("p (t c) -> p t c", c=8)
                        .unsqueeze(3).broadcast_to([128, NT, 8, K]),
                    op=mybir.AluOpType.is_equal,
                )
                cand = workp.tile([128, TOK], F32, tag="cand", name=f"cand{j}")
                nc.vector.scalar_tensor_tensor(
                    out=cand[:, :].rearrange("p (t f) -> p t f", f=128),
                    in0=eq[:, :].rearrange("p (t f) -> p t f", f=128),
                    in1=iota_sb[:, :].unsqueeze(1)
                        .broadcast_to([128, NT, 128]),
                    scalar=-BIG,
                    op0=mybir.AluOpType.mult, op1=mybir.AluOpType.add,
                )
                nc.vector.tensor_reduce(
                    out=kminbig[:, :].rearrange(
                        "p (t c) -> p t c", c=NC)[:, :, 8 * j:8 * (j + 1)],
                    in_=cand[:, :].rearrange("p (t c k) -> p t c k", k=K, c=8),
                    axis=mybir.AxisListType.X, op=mybir.AluOpType.min,
                )

            def ship_half(half):
                # kmin [n, c-half] -> bf16 -> transpose -> AllGather.
                # Collective order (AR, AG1, AG2) matches input readiness so
                # the FIFO collective queue never head-of-line blocks.
                c0 = (NC // 2) * half
                kth = constp.tile([NC // 2, TOK], BF16, tag=f"kt_sb{half}",
                                  name=f"kt_sb{half}")
                for t in range(NT):
                    kminb = workp.tile([128, NC // 2], BF16, tag="kminb",
                                       name=f"kminb{half}_{t}")
                    nc.vector.tensor_scalar_add(
                        kminb[:, :].rearrange("p (c j) -> p c j", j=4),
                        kminbig[:, NC * t + c0:NC * t + c0 + NC // 2]
                            .rearrange("p (j c) -> p j c", c=8)
                            .transpose([0, 2, 1]),
                        BIG)
                    ps_kt = psp.tile([NC // 2, 128], BF16, tag="ps",
                                     name=f"ps_kt{half}_{t}")
                    nc.tensor.transpose(ps_kt[:, :], kminb[:, :],
                                        identb[:, :])
                    nc.vector.tensor_copy(
                        out=kth[:, 128 * t:128 * (t + 1)], in_=ps_kt[:, :])
                bnc = kt_bounce1 if half == 0 else kt_bounce2
                gat = kt_all1 if half == 0 else kt_all2
                nc.gpsimd.dma_start(out=bnc[:, :], in_=kth[:, :])
                nc.gpsimd.collective_compute(
                    "AllGather",
                    mybir.AluOpType.bypass,
                    replica_groups=[list(range(N_CORES))],
                    ins=[bnc.opt()],
                    outs=[gat.opt()],
                )

            for j in range(NCHUNK):
                ps_xct = psp.tile([128, TOK], F32, tag="ps", name=f"ps_xct{j}")
                for mcol in range(4):
                    p = 4 * j + mcol
                    xt_t = xtp.tile([128, 2 * TOK], BF16, tag="xt")
                    nc.sync.dma_start(out=xt_t[:, :],
                                      in_=xhl.ap()[128 * p:128 * (p + 1), :])
                    passes = [(cbdh_sb, 0), (cbdh_sb, TOK), (cbdl_sb, 0)]
                    for i, (cb, off) in enumerate(passes):
                        nc.tensor.matmul(
                            ps_xct[32 * mcol:32 * (mcol + 1), :],
                            lhsT=cb[:, 32 * p:32 * (p + 1)],
                            rhs=xt_t[:, off:off + TOK],
                            start=(i == 0), stop=(i == 2),
                            tile_position=(0, 32 * mcol),
                        )
                xct_sb = xctp.tile([128, TOK], F32, tag="xct")
                nc.vector.tensor_copy(out=xct_sb[:, :], in_=ps_xct[:, :])
                ps_tr4 = psp.tile([128, TOK], F32, tag="ps", name=f"ps_tr4_{j}")
                for t in range(NT):
                    nc.tensor.transpose(
                        ps_tr4[:, 128 * t:128 * (t + 1)],
                        xct_sb[:, 128 * t:128 * (t + 1)],
                        identf[:, :],
                    )
                ps_trs[j] = ps_tr4
                if j > 0:
                    argmin_chunk(j - 1)
                    ps_trs[j - 1] = None
                if j == 4:
                    ship_half(0)
            argmin_chunk(NCHUNK - 1)
            ship_half(1)

            # gmax recovery AFTER the AG triggers: the gpsimd stream must not
            # stall on the AllReduce wait before issuing the AllGathers.
            gmax_row = constp.tile([1, 16], F32, tag="gmax_row")
            nc.gpsimd.dma_start(out=gmax_row[:, :], in_=mx_out[:, :])
            gmax = constp.tile([128, 1], F32, tag="gmax")
            nc.gpsimd.partition_broadcast(gmax[:, :], gmax_row[0:1, 0:1])
            # s = gmax/127 and 1/s via reciprocal (DVE has no divide); the
            # <=2ulp drift is far inside the error budget.
            s_col = constp.tile([128, 1], F32, tag="s_col")
            nc.vector.tensor_scalar(
                out=s_col[:, :], in0=gmax[:, :],
                scalar1=float(np.float32(1.0) / np.float32(127.0)),
                scalar2=None, op0=mybir.AluOpType.mult,
            )
            rgmax = constp.tile([128, 1], F32, tag="rgmax")
            nc.vector.reciprocal(rgmax[:, :], gmax[:, :])
            inv_s = constp.tile([128, 1], F32, tag="inv_s")
            nc.vector.tensor_scalar(
                out=inv_s[:, :], in0=rgmax[:, :], scalar1=127.0, scalar2=None,
                op0=mybir.AluOpType.mult,
            )

            # ---- phase 5: quantize-dequantize lut -> bf16 ------------------
            lutdq = lutp.tile([128, NCHUNK * OSH], BF16, tag="lutdq")
            for j in range(NCHUNK):
                qm = workp.tile([128, OSH], F32, tag="qm")
                nc.vector.tensor_scalar(
                    out=qm[:, :], in0=lut_sb[:, OSH * j:OSH * (j + 1)],
                    scalar1=inv_s[:, 0:1], scalar2=MAGIC,
                    op0=mybir.AluOpType.mult, op1=mybir.AluOpType.add,
                )
                nc.vector.tensor_scalar(
                    out=lutdq[:, OSH * j:OSH * (j + 1)], in0=qm[:, :],
                    scalar1=MAGIC, scalar2=s_col[:, 0:1],
                    op0=mybir.AluOpType.subtract, op1=mybir.AluOpType.mult,
                )

            bias_sb = constp.tile([128, OSH], F32, tag="bias_sb")
            nc.sync.dma_start(out=bias_sb[:, :], in_=biasr.ap()[:, :])
            kcol_sb = constp.tile([128, 1], F32, tag="kcol_sb")
            nc.sync.dma_start(out=kcol_sb[:, :], in_=kcol.ap()[:, :])

            # ---- phase 6: expand one-hots + gather matmul ------------------
            # Two half-contractions: chunks 0-3 right after AG1 (overlapping
            # AG2), chunks 4-7 after AG2; bias folded into the first half.
            # One 4D-AP replication DMA + one is_equal per (half, rank).
            half_tiles = {}
            for half in range(2):
                gat = kt_all1 if half == 0 else kt_all2
                for r in range(N_CORES):
                    rep = strepp.tile([128, 4 * TOK], BF16, tag="strep")
                    nc.scalar.dma_start(
                        out=rep[:, :],
                        in_=gat[32 * r:32 * (r + 1), :]
                            .rearrange("(c j) n -> c (j n)", j=4)
                            .unsqueeze(1).broadcast_to([8, K, 4 * TOK]),
                    )
                    stg_t = stgp.tile([128, 4 * TOK], BF16, tag="stg")
                    nc.vector.tensor_scalar(
                        out=stg_t[:, :], in0=rep[:, :],
                        scalar1=kcol_sb[:, 0:1], scalar2=None,
                        op0=mybir.AluOpType.is_equal,
                    )
                    o_sb = None
                    if half == 1:
                        o_sb = outp.tile([128, NT * OSH], BF16, tag="outsb")
                    for t in range(NT):
                        ps_o = psp.tile([128, OSH], F32, tag="ps")
                        for jj in range(NCHUNK // 2):
                            jglob = 4 * half + jj
                            nc.tensor.matmul(
                                ps_o[:, :],
                                lhsT=stg_t[:, TOK * jj + 128 * t:
                                           TOK * jj + 128 * (t + 1)],
                                rhs=lutdq[:, OSH * jglob:OSH * (jglob + 1)],
                                start=(jj == 0), stop=(jj == NCHUNK // 2 - 1),
                            )
                        if half == 0:
                            hs = halfp.tile([128, OSH], BF16, tag="hsum",
                                            name=f"hs{r}_{t}")
                            nc.vector.tensor_tensor(
                                out=hs[:, :], in0=ps_o[:, :], in1=bias_sb[:, :],
                                op=mybir.AluOpType.add,
                            )
                            half_tiles[(r, t)] = hs
                        else:
                            nc.vector.tensor_tensor(
                                out=o_sb[:, OSH * t:OSH * (t + 1)],
                                in0=ps_o[:, :], in1=half_tiles[(r, t)][:, :],
                                op=mybir.AluOpType.add,
                            )
                    if half == 1:
                        nc.sync.dma_start(
                            out=out.ap()[TOK * r:TOK * (r + 1), :]
                                .rearrange("(t p) o -> p t o", p=128),
                            in_=o_sb[:, :].rearrange("p (t o) -> p t o", o=OSH),
                        )

    nc.compile()
    return nc


def _prep_inputs(x, centroids, weight, bias):
    import ml_dtypes

    x = np.ascontiguousarray(np.asarray(x, dtype=np.float32)).reshape(BN, IN_F)
    cent = np.asarray(centroids, dtype=np.float32)
    w = np.asarray(weight, dtype=np.float32)
    bias = np.asarray(bias, dtype=np.float32)

    c2 = (cent ** 2).sum(axis=-1).reshape(CK)  # [1024] flat (c,k)
    c2r = np.ascontiguousarray(np.broadcast_to(c2, (128, CK)))
    iota = np.tile(np.arange(K, dtype=np.float32), 8)
    iotar = np.ascontiguousarray(np.broadcast_to(iota, (128, 128)))
    cbd = np.zeros((128, CK), np.float32)
    for p in range(NPAIR):
        cbd[0:SUBV, 32 * p:32 * p + K] = cent[2 * p].T
        cbd[SUBV:128, 32 * p + K:32 * p + 2 * K] = cent[2 * p + 1].T
    cbd_h = cbd.astype(ml_dtypes.bfloat16)
    cbd_l = (cbd - cbd_h.astype(np.float32)).astype(ml_dtypes.bfloat16)
    cbd_f16 = cbd.astype(np.float16)
    kcol = np.ascontiguousarray(
        (np.arange(128, dtype=np.float32) % K).reshape(128, 1))

    in_maps = []
    for r in range(N_CORES):
        xT_r = np.ascontiguousarray(x[TOK * r:TOK * (r + 1)].T)
        xh_r = xT_r.astype(ml_dtypes.bfloat16)
        xl_r = (xT_r - xh_r.astype(np.float32)).astype(ml_dtypes.bfloat16)
        xhl_r = np.ascontiguousarray(np.concatenate([xh_r, xl_r], axis=1))
        w_r = np.ascontiguousarray(w[:, :, OSH * r:OSH * (r + 1)]).reshape(
            IN_F, OSH)
        wf16_r = w_r.astype(np.float16)
        bias_r = np.ascontiguousarray(
            np.broadcast_to(bias[OSH * r:OSH * (r + 1)], (128, OSH)))
        in_maps.append({
            "xhl": xhl_r, "wf16": wf16_r, "cbdf16": cbd_f16,
            "cbd_h": cbd_h, "cbd_l": cbd_l,
            "c2r": c2r, "iotar": iotar, "biasr": bias_r, "kcol": kcol,
        })
    return in_maps


def kernel(x, centroids, weight, inverse_temperature_logit, bias,
           **_unused) -> np.ndarray:
    if "nc" not in _CACHE:
        _CACHE["nc"] = _build()
    nc = _CACHE["nc"]
    in_maps = _prep_inputs(x, centroids, weight, bias)
    res = run_bass_kernel_spmd(nc, in_maps, core_ids=list(range(N_CORES)))
    out = np.concatenate(
        [res.results[r]["out"].astype(np.float32) for r in range(N_CORES)],
        axis=1)
    return out.reshape(2, BN // 2, OUT_F)
